# revision 20
# baseline (speedup 1.0000x reference)
"""DNABERT2 (4-layer BERT w/ ALiBi + GEGLU) forward pass on 8 Trainium2 cores.

Strategy: sequence-parallel over the 2048 tokens (256 tokens/core).
 - Residual stream x kept FEATURE-MAJOR in SBUF: [128 part, 6 ftile, 256 tok] fp32.
 - All matmul operands bf16 (weights cast on host; activations produced bf16).
 - Attention: scores computed TRANSPOSED ([key, query]) so the ALiBi column
   bias (slope_h * (j - (L-1)) + pad_j) is a per-partition ACT bias fused into
   the Exp op.  Softmax uses the fixed shift slope_h*(L-1) instead of a row
   max (shift-invariance; the q.k part is O(5) so exp cannot overflow).
 - PV matmul uses v-hat = [v | ones] per head (65 columns) so row 64 of the
   PV accumulator is the softmax denominator; normalization happens via
   reciprocal + PE ones-outer-product broadcast.
 - Per layer one packed AllGather ships each core's K/V shard (bf16) to all.
 - MLM head emits token-major [256, 4096] logits per core; host concatenates.
"""
import sys, math, os
sys.path.insert(0, "/opt/trn_rl_repo")

import numpy as np
import ml_dtypes

import concourse.bass as bass
import concourse.bacc as bacc
import concourse.tile as tile
from concourse import mybir
from concourse import bass_utils

AF = mybir.ActivationFunctionType
ALU = mybir.AluOpType
BF16 = mybir.dt.float16  # "half" dtype for matmul operands (fp16: 10-bit mantissa)
FP8 = mybir.dt.float8e4  # e4m3: K/V/attn-weight storage + AllGather payload
F32 = mybir.dt.float32
I32 = mybir.dt.int32

# model dims
V = 4096; E = 768; H = 12; NL = 4; FFN = 2048; B = 1; L = 2048; HD = 64
EPS = 1e-5
NC = 8            # cores
T = L // NC       # tokens per core = 256
TT = T // 128     # token tiles per core = 2
F6 = E // 128     # feature tiles = 6
KT = L // 128     # key tiles = 16
SCALE = 1.0 / math.sqrt(HD)

# config knobs
SKIP_THRESH = float(os.environ.get("KERN_SKIP_THRESH", "12"))  # 0 = no tile skipping
TRACE = os.environ.get("KERN_TRACE", "0") == "1"
NO_AG = os.environ.get("KERN_NO_AG", "0") == "1"  # timing-only: skip collective
DEBUG = os.environ.get("KERN_DEBUG", "0") == "1"


def _alibi_slopes(n):
    def pow2(m):
        start = 2.0 ** (-2.0 ** (-(math.log2(m) - 3)))
        return [start * start ** i for i in range(m)]
    if math.log2(n).is_integer():
        return np.array(pow2(n), dtype=np.float32)
    c = 2 ** math.floor(math.log2(n))
    s = pow2(c) + pow2(2 * c)[0::2][: n - c]
    return np.array(s, dtype=np.float32)

SLOPES = _alibi_slopes(H)  # (12,)


def kept_ktiles(h):
    """Key tiles whose max ALiBi bias is within SKIP_THRESH of the top;
    others underflow in the softmax and are skipped entirely."""
    if SKIP_THRESH <= 0:
        return list(range(KT))
    s = float(SLOPES[h])
    keep = [kt for kt in range(KT)
            if s * ((L - 1) - (kt * 128 + 127)) < SKIP_THRESH]
    return keep if keep else [KT - 1]


def bf(a):
    return np.ascontiguousarray(a).astype(np.float16)


# ---------------------------------------------------------------------------
# device program
# ---------------------------------------------------------------------------

_CACHE = {}


def build(flags, reps=1):
    key = (flags, reps)
    if key in _CACHE:
        return _CACHE[key]
    (ln_affine, has_qkb, has_vb, has_ob, has_f1b, has_f2b, has_m1b,
     has_m2b) = flags

    nc = bacc.Bacc("TRN2", target_bir_lowering=False, debug=False,
                   num_devices=NC)

    # ---- dram tensors ----
    tok_d = nc.dram_tensor("tok", [T, 1], I32, kind="ExternalInput")
    emb_d = nc.dram_tensor("emb", [V, E], F32, kind="ExternalInput")
    id_d = nc.dram_tensor("ident", [128, 128], F32, kind="ExternalInput")
    btab_d = nc.dram_tensor("btab", [128, KT * H], F32, kind="ExternalInput")
    wqk_d = [nc.dram_tensor(f"wqk{i}", [E, 2 * E], BF16, kind="ExternalInput") for i in range(NL)]
    wv_d = [nc.dram_tensor(f"wv{i}", [E, E], BF16, kind="ExternalInput") for i in range(NL)]
    wo_d = [nc.dram_tensor(f"wo{i}", [E, E], BF16, kind="ExternalInput") for i in range(NL)]
    w1_d = [nc.dram_tensor(f"w1{i}", [E, 2 * FFN], BF16, kind="ExternalInput") for i in range(NL)]
    w2_d = [nc.dram_tensor(f"w2{i}", [FFN, E], BF16, kind="ExternalInput") for i in range(NL)]
    mw1_d = nc.dram_tensor("mw1", [E, E], BF16, kind="ExternalInput")
    mw2_d = nc.dram_tensor("mw2", [E, V], BF16, kind="ExternalInput")
    # ln params packed [128, 6] each, one tensor [128, n*6]
    n_ln = 2 * NL + 2
    lng_d = nc.dram_tensor("lng", [128, n_ln * F6], F32, kind="ExternalInput")
    lnb_d = nc.dram_tensor("lnb", [128, n_ln * F6], F32, kind="ExternalInput")
    # linear biases (always declared; host passes zeros; ops emitted per flag)
    qkb_d = nc.dram_tensor("qkb", [128, 2 * F6], F32, kind="ExternalInput")
    vb_d = nc.dram_tensor("vb", [1, E], F32, kind="ExternalInput")
    ob_d = nc.dram_tensor("ob", [128, F6], F32, kind="ExternalInput")
    f1b_d = nc.dram_tensor("f1b", [128, 2 * FFN // 128], F32, kind="ExternalInput")
    f2b_d = nc.dram_tensor("f2b", [128, F6], F32, kind="ExternalInput")
    m1b_d = nc.dram_tensor("m1b", [128, F6], F32, kind="ExternalInput")
    m2b_d = nc.dram_tensor("m2b", [1, V], F32, kind="ExternalInput")
    out_d = nc.dram_tensor("out", [T, V], BF16, kind="ExternalOutput")
    dbg_d = {}
    if DEBUG:
        for nm, shape in [("x0", [128, F6 * T]), ("h0", [128, F6 * T]),
                          ("q0", [128, F6 * T]), ("k0", [128, F6 * T]),
                          ("v0", [128, TT * E]), ("ksb", [128, F6 * 128]),
                          ("vhb", [128, 2 * 780]), ("at0", [128, 2 * T]),
                          ("o0", [128, F6 * T]), ("x1", [128, F6 * T]),
                          ("sums", [H, T]), ("rrs", [H, T]),
                          ("ath", [128, H * T]), ("sch", [128, H * T]),
                          ("or1", [65, T]), ("bc1", [64, T]),
                          ("z0", [128, T]), ("x2", [128, F6 * T])]:
            dbg_d[nm] = nc.dram_tensor("dbg_" + nm, shape, F32,
                                       kind="ExternalOutput")

    KELEM = 128 * F6 * T          # k shard elems per core (196608, fp8)
    VELEM = T * E                 # v shard elems per core (196608, fp16)
    AGE = KELEM + 2 * VELEM       # AG payload bytes (fp8-elem units)

    from contextlib import ExitStack
    with tile.TileContext(nc) as tc, ExitStack() as _es:
        constp = _es.enter_context(tc.tile_pool(name="const", bufs=1))
        residp = _es.enter_context(tc.tile_pool(name="resid", bufs=1))
        actp = _es.enter_context(tc.tile_pool(name="act", bufs=2))
        kvp = _es.enter_context(tc.tile_pool(name="kv", bufs=1))
        attnp = _es.enter_context(tc.tile_pool(name="attn", bufs=1))
        smallp = _es.enter_context(tc.tile_pool(name="small", bufs=2))
        wpool = _es.enter_context(tc.tile_pool(name="wpool", bufs=6))
        w2pool = _es.enter_context(tc.tile_pool(name="w2pool", bufs=2))
        mw2pool = _es.enter_context(tc.tile_pool(name="mw2pool", bufs=1))
        zpool = _es.enter_context(tc.tile_pool(name="zpool", bufs=16))
        psmm = _es.enter_context(tc.tile_pool(name="psmm", bufs=2, space="PSUM"))
        pssc = _es.enter_context(tc.tile_pool(name="pssc", bufs=2, space="PSUM"))
        pspv = _es.enter_context(tc.tile_pool(name="pspv", bufs=2, space="PSUM"))
        psh = _es.enter_context(tc.tile_pool(name="psh", bufs=2, space="PSUM"))
        dramp = _es.enter_context(tc.tile_pool(name="dram", bufs=2, space="DRAM"))

        # ---- constants ----
        ident = constp.tile([128, 128], F32)
        nc.sync.dma_start(out=ident, in_=id_d.ap())
        btab = constp.tile([128, KT * H], F32)
        nc.sync.dma_start(out=btab, in_=btab_d.ap())
        inv_e_bf = constp.tile([128, 1], BF16)
        nc.vector.memset(inv_e_bf, 1.0 / E)
        ones_f = constp.tile([128, 128], F32)
        nc.vector.memset(ones_f, 1.0)
        ones_row_bf = constp.tile([1, 128], BF16)
        nc.vector.memset(ones_row_bf, 1.0)
        eps_t = constp.tile([128, 1], F32)
        nc.vector.memset(eps_t, EPS)
        lng = constp.tile([128, n_ln * F6], F32)
        nc.sync.dma_start(out=lng, in_=lng_d.ap())
        lnb = constp.tile([128, n_ln * F6], F32)
        nc.sync.dma_start(out=lnb, in_=lnb_d.ap())
        qkb = constp.tile([128, 2 * F6], F32)
        nc.sync.dma_start(out=qkb, in_=qkb_d.ap())
        vb = constp.tile([1, E], F32)
        nc.sync.dma_start(out=vb, in_=vb_d.ap())
        ob = constp.tile([128, F6], F32)
        nc.sync.dma_start(out=ob, in_=ob_d.ap())
        f1b = constp.tile([128, 2 * FFN // 128], F32)
        nc.sync.dma_start(out=f1b, in_=f1b_d.ap())
        f2b = constp.tile([128, F6], F32)
        nc.sync.dma_start(out=f2b, in_=f2b_d.ap())
        m1b = constp.tile([128, F6], F32)
        nc.sync.dma_start(out=m1b, in_=m1b_d.ap())
        m2b = constp.tile([1, V], F32)
        nc.sync.dma_start(out=m2b, in_=m2b_d.ap())

        def tap(nm, ap):
            if DEBUG and nm in dbg_d:
                shp = list(ap.shape)
                mid = int(np.prod(shp[1:-1])) if len(shp) > 2 else 1
                pad = shp[:-1] + [max(shp[-1], -(-(F6 * T) // mid))]
                t = smallp.tile(shp, F32, tag="dbgscratch", bufs=1,
                                padded_shape=pad)
                nc.vector.tensor_copy(out=t, in_=ap)
                nc.sync.dma_start(out=dbg_d[nm].ap().rearrange(
                    "p (a b) -> p a b", a=ap.shape[1]) if len(ap.shape) == 3
                    else dbg_d[nm].ap(), in_=t)

        # ---- residual stream ----
        x = residp.tile([128, F6, T], F32)

        for _rep in range(reps):
            # ---- embedding gather + transpose to feature-major ----
            tokt = constp.tile([128, TT], I32)
            nc.sync.dma_start(out=tokt,
                              in_=tok_d.ap().rearrange("(a p) o -> p (a o)", p=128))
            for tt in range(TT):
                xtm = actp.tile([128, E], F32, tag="xtm", bufs=1)
                nc.gpsimd.indirect_dma_start(
                    out=xtm[:], out_offset=None, in_=emb_d.ap(),
                    in_offset=bass.IndirectOffsetOnAxis(ap=tokt[:, tt:tt + 1], axis=0))
                for f in range(F6):
                    tp = psmm.tile([128, 128], F32, tag="mm")
                    nc.tensor.transpose(out=tp[:], in_=xtm[:, f * 128:(f + 1) * 128],
                                        identity=ident[:])
                    nc.vector.tensor_copy(out=x[:, f, tt * 128:(tt + 1) * 128],
                                          in_=tp[:])

            # ---------------- helpers ----------------
            def layernorm(src, h_dst, ln_idx):
                """src fp32 [128,F6,T] -> h_dst bf16 [128,F6,T] (LN over features)."""
                g_ap = lng[:, ln_idx * F6:(ln_idx + 1) * F6]
                b_ap = lnb[:, ln_idx * F6:(ln_idx + 1) * F6]
                # cat holds [x | x^2] side by side so one matmul per ftile
                # produces both sums; inv_e lhsT folds the 1/E scaling.
                cat = kvp.tile([128, F6, 2, T], BF16, tag="ln_xb")
                nc.vector.tensor_copy(out=cat[:, :, 0, :], in_=src)
                nc.vector.tensor_tensor(out=cat[:, :, 1, :], in0=cat[:, :, 0, :],
                                        in1=cat[:, :, 0, :], op=ALU.mult)
                psAB = pssc.tile([1, 2 * T], F32, tag="sc")
                for k in range(F6):
                    nc.tensor.matmul(psAB[:], inv_e_bf[:, 0:1], cat[:, k, :, :],
                                     start=(k == 0), stop=(k == F6 - 1))
                st = smallp.tile([1, 4 * T], F32, tag="ln_st", bufs=1)
                # slots: 0 m, 1 ms/var/sqrt, 2 rstd, 3 mrstd  (2,3 adjacent for bcast)
                sl = lambda i: st[:, i * T:(i + 1) * T]
                nc.vector.tensor_copy(out=st[:, 0:2 * T], in_=psAB[:])
                nc.vector.tensor_tensor(out=sl(3), in0=sl(0), in1=sl(0), op=ALU.mult)
                nc.vector.tensor_tensor(out=sl(1), in0=sl(1), in1=sl(3),
                                        op=ALU.subtract)          # var
                nc.scalar.activation(out=sl(1), in_=sl(1), func=AF.Sqrt,
                                     bias=eps_t[0:1, 0:1])
                nc.vector.reciprocal_approx_fast(out=sl(2), in_=sl(1))   # rstd
                nc.vector.tensor_tensor(out=sl(3), in0=sl(0), in1=sl(2), op=ALU.mult)
                bc = pssc.tile([128, 2 * T], F32, tag="sc")
                nc.tensor.matmul(bc[:], ones_f[0:1, 0:128], st[0:1, 2 * T:4 * T],
                                 start=True, stop=True)
                for f in range(F6):
                    tmp = smallp.tile([128, T], F32, tag="ln_tmp")
                    nc.vector.tensor_tensor(out=tmp, in0=src[:, f, :],
                                            in1=bc[:, 0:T], op=ALU.mult)
                    nc.vector.tensor_tensor(out=h_dst[:, f, :], in0=tmp,
                                            in1=bc[:, T:2 * T], op=ALU.subtract)
                    if ln_affine:
                        nc.vector.tensor_scalar(
                            out=h_dst[:, f, :], in0=h_dst[:, f, :],
                            scalar1=g_ap[:, f:f + 1], scalar2=b_ap[:, f:f + 1],
                            op0=ALU.mult, op1=ALU.add)

            def linear_fm(h_bf, w_dram, mtiles, out_cb, bias_tile=None,
                          w_tag="w"):
                """out[m] = W.T-tile-m @ h (feature-major out), psum -> out_cb."""
                for m in range(mtiles):
                    wt = wpool.tile([128, F6, 128], BF16, tag=w_tag)
                    nc.sync.dma_start(
                        out=wt,
                        in_=w_dram.ap().rearrange("(a p) n -> p a n", p=128)[
                            :, :, m * 128:(m + 1) * 128])
                    ps = psmm.tile([128, T], F32, tag="mm")
                    for k in range(F6):
                        nc.tensor.matmul(ps[:], wt[:, k, :], h_bf[:, k, :],
                                         start=(k == 0), stop=(k == F6 - 1))
                    out_cb(m, ps, bias_tile)

            tap("x0", x[:])
            # ---------------- layers ----------------
            for i in range(NL):
                h_bf = actp.tile([128, F6, T], BF16, tag="h")
                layernorm(x, h_bf, 2 * i)
                if i == 0:
                    tap("h0", h_bf[:])

                # --- qkv: K first so its AllGather launches ASAP ---
                q_bf = actp.tile([128, F6, T], BF16, tag="q", bufs=1)
                kshard = actp.tile([128, F6, T], FP8, tag="kshard", bufs=1)
                wqkr = wqk_d[i].ap().rearrange("(a p) n -> p a n", p=128)

                def qk_mm(m, dst):
                    wt = wpool.tile([128, F6, 128], BF16, tag="w")
                    nc.sync.dma_start(out=wt,
                                      in_=wqkr[:, :, m * 128:(m + 1) * 128])
                    ps = psmm.tile([128, T], F32, tag="mm")
                    for k in range(F6):
                        nc.tensor.matmul(ps[:], wt[:, k, :], h_bf[:, k, :],
                                         start=(k == 0), stop=(k == F6 - 1))
                    if has_qkb:
                        nc.vector.tensor_scalar_add(out=dst, in0=ps[:],
                                                    scalar1=qkb[:, m:m + 1])
                    else:
                        nc.vector.tensor_copy(out=dst, in_=ps[:])

                for m in range(F6, 2 * F6):
                    qk_mm(m, kshard[:, m - F6, :])

                ag_in = dramp.tile([1, AGE], FP8, tag="ag_in")
                ag_out = dramp.tile([NC, AGE], FP8, tag="ag_out",
                                    addr_space="Shared")
                nc.sync.dma_start(
                    out=ag_in[0:1, 0:KELEM].rearrange("o (p f t) -> (o p) f t",
                                                      p=128, f=F6),
                    in_=kshard)

                # --- V projection (packed into the same AllGather as K) ---
                vshard = actp.tile([128, TT, E], BF16, tag="vshard", bufs=1)
                wvt = kvp.tile([128, F6, E], BF16, tag="wv")
                nc.sync.dma_start(out=wvt,
                                  in_=wv_d[i].ap().rearrange("(a p) n -> p a n", p=128))
                for tt in range(TT):
                    for n0 in (0, 384):
                        ps = psmm.tile([128, 384], F32, tag="mm")
                        for k in range(F6):
                            nc.tensor.matmul(ps[:], h_bf[:, k, tt * 128:(tt + 1) * 128],
                                             wvt[:, k, n0:n0 + 384],
                                             start=(k == 0), stop=(k == F6 - 1))
                        if has_vb:
                            bcv = pssc.tile([128, 384], F32, tag="sc")
                            nc.tensor.matmul(bcv[:], ones_f[0:1, :],
                                             vb[0:1, n0:n0 + 384], start=True, stop=True)
                            nc.vector.tensor_tensor(out=vshard[:, tt, n0:n0 + 384],
                                                    in0=ps[:], in1=bcv[:], op=ALU.add)
                        else:
                            nc.vector.tensor_copy(out=vshard[:, tt, n0:n0 + 384],
                                                  in_=ps[:])
                if i == 0:
                    tap("v0", vshard[:])
                nc.sync.dma_start(
                    out=ag_in[0:1, KELEM:AGE].bitcast(BF16).rearrange(
                        "o (tt p e) -> (o p) tt e", tt=TT, p=128),
                    in_=vshard)
                if NO_AG:
                    nc.sync.dma_start(out=ag_out[0:1, :], in_=ag_in[:])
                else:
                    nc.gpsimd.collective_compute(
                        "AllGather", ALU.bypass,
                        replica_groups=[list(range(NC))],
                        ins=[ag_in[:]], outs=[ag_out[:]])
                k_sb = kvp.tile([128, F6, L], FP8, tag="k_sb")
                # load only the key-tile tail ranges any head of this
                # feature-pair keeps (ALiBi tile skipping)
                ksrc = ag_out[:, 0:KELEM].rearrange("r (p f t) -> p f r t",
                                                    p=128, f=F6)
                kdst = k_sb.rearrange("p f (r t) -> p f r t", r=NC)
                for f0 in range(F6):
                    kmin = min(kept_ktiles(2 * f0)[0], kept_ktiles(2 * f0 + 1)[0])
                    r0 = kmin // TT
                    nc.sync.dma_start(out=kdst[:, f0:f0 + 1, r0:NC, :],
                                      in_=ksrc[:, f0:f0 + 1, r0:NC, :])
                vhat = kvp.tile([128, KT, H * 65], BF16, tag="vhat")
                VROW = H * 65  # 780 elems per (partition, ktile)
                for kt in range(KT):
                    r, tt = kt // TT, kt % TT
                    off = tt * (128 * E)
                    # heads that keep this key tile, as contiguous runs
                    hs = [h for h in range(H) if kept_ktiles(h)[0] <= kt]
                    runs = []
                    for h in hs:
                        if runs and runs[-1][1] == h - 1:
                            runs[-1][1] = h
                        else:
                            runs.append([h, h])
                    vsrc = ag_out[r:r + 1, KELEM + 2 * off:
                                  KELEM + 2 * (off + 128 * E)].bitcast(
                        BF16).rearrange("r (p h d) -> (r p) h d", p=128, h=H)
                    for h0, h1 in runs:
                        nh = h1 - h0 + 1
                        nc.sync.dma_start(
                            out=bass.AP(tensor=vhat.tensor,
                                        offset=vhat.offset + kt * VROW + h0 * 65,
                                        ap=[vhat.ap[0],
                                            [65, nh],           # h
                                            [1, 64]]),          # d
                            in_=vsrc[:, h0:h1 + 1, :])
                # ones columns (col 64 of each head block)
                nc.vector.memset(
                    bass.AP(tensor=vhat.tensor, offset=vhat.offset + 64,
                            ap=[vhat.ap[0], [VROW, KT], [65, H]]),
                    1.0)

                # --- Q projection (overlaps the AllGathers) ---
                for m in range(F6):
                    qk_mm(m, q_bf[:, m, :])
                if i == 0:
                    tap("q0", q_bf[:])
                    tap("k0", kshard[:])
                if i == 0:
                    tap("ksb", k_sb[:, :, 0:128])
                    tap("vhb", vhat[:, 0:2, :])
                # --- attention: QK+exp for ALL heads first (overlaps the V
                # AllGather), then all PV+normalize ---
                o_bf = actp.tile([128, F6, T], BF16, tag="o", bufs=1)
                attnTs = []
                for h in range(H):
                    f0, r0 = h // 2, (h % 2) * 64
                    kts = kept_ktiles(h)
                    attnT = attnp.tile([128, len(kts), T], BF16,
                                       tag=f"attnT{h}", name=f"attnT{h}")
                    for g0 in range(0, len(kts), 2):
                        grp = kts[g0:g0 + 2]
                        sc = pssc.tile([128, len(grp) * T], F32, tag="sc")
                        for j, kt in enumerate(grp):
                            nc.tensor.matmul(
                                sc[:, j * T:(j + 1) * T],
                                k_sb[r0:r0 + 64, f0, kt * 128:(kt + 1) * 128],
                                q_bf[r0:r0 + 64, f0, :],
                                start=True, stop=True)
                            if DEBUG and i == 0 and kt == KT - 1:
                                scht = smallp.tile([128, T], F32, tag="scht")
                                nc.vector.tensor_copy(out=scht,
                                                      in_=sc[:, j * T:(j + 1) * T])
                                nc.sync.dma_start(
                                    out=dbg_d["sch"].ap()[:, h * T:(h + 1) * T],
                                    in_=scht)
                            nc.scalar.activation(
                                out=attnT[:, g0 + j, :], in_=sc[:, j * T:(j + 1) * T],
                                func=AF.Exp, bias=btab[:, kt * H + h:kt * H + h + 1],
                                scale=SCALE)
                    if DEBUG and i == 0:
                        atht = smallp.tile([128, T], F32, tag="atht")
                        nc.vector.tensor_copy(out=atht, in_=attnT[:, len(kts) - 1, :])
                        nc.sync.dma_start(out=dbg_d["ath"].ap()[:, h * T:(h + 1) * T],
                                          in_=atht)
                    attnTs.append((attnT, kts))
                for h in range(H):
                    f0, r0 = h // 2, (h % 2) * 64
                    attnT, kts = attnTs[h]
                    if i == 0 and h == 0 and len(kts) >= 2 and kts[0] == 0:
                        tap("at0", attnT[:, 0:2, :])
                    pv = pspv.tile([65, T], F32, tag="pv")
                    for n, kt in enumerate(kts):
                        nc.tensor.matmul(pv[:], vhat[:, kt, h * 65:h * 65 + 65],
                                         attnT[:, n, :],
                                         start=(n == 0), stop=(n == len(kts) - 1))
                    o_raw = smallp.tile([65, T], F32, tag="o_raw")
                    nc.scalar.activation(out=o_raw, in_=pv[:], func=AF.Copy)
                    srow = smallp.tile([1, T], F32, tag="srow")
                    nc.vector.tensor_copy(out=srow[0:1, :], in_=o_raw[64:65, :])
                    rr = smallp.tile([1, T], F32, tag="rr")
                    nc.vector.reciprocal_approx_fast(out=rr[0:1, :],
                                                     in_=srow[0:1, :])
                    bc = psmm.tile([64, T], F32, tag="mm")
                    nc.tensor.matmul(bc[:], ones_f[0:1, 0:64], rr[0:1, :],
                                     start=True, stop=True)
                    if DEBUG and i == 0:
                        nc.sync.dma_start(out=dbg_d["sums"].ap()[h:h + 1, :],
                                          in_=o_raw[64:65, :])
                        nc.sync.dma_start(out=dbg_d["rrs"].ap()[h:h + 1, :],
                                          in_=rr[0:1, :])
                        if h == 1:
                            nc.sync.dma_start(out=dbg_d["or1"].ap(), in_=o_raw[:])
                            bc1t = smallp.tile([64, T], F32, tag="bc1t", bufs=1)
                            nc.vector.tensor_copy(out=bc1t, in_=bc[:])
                            nc.sync.dma_start(out=dbg_d["bc1"].ap(), in_=bc1t)
                    nc.vector.tensor_tensor(out=o_bf[r0:r0 + 64, f0, :],
                                            in0=o_raw[0:64, :], in1=bc[:],
                                            op=ALU.mult)
                # --- out_proj + residual ---
                def oproj_cb(m, ps, _):
                    if has_ob:
                        tmp2 = smallp.tile([128, T], F32, tag="ob_tmp")
                        nc.vector.tensor_scalar_add(out=tmp2, in0=ps[:],
                                                    scalar1=ob[:, m:m + 1])
                        nc.vector.tensor_tensor(out=x[:, m, :], in0=x[:, m, :],
                                                in1=tmp2, op=ALU.add)
                    else:
                        nc.vector.tensor_tensor(out=x[:, m, :], in0=x[:, m, :],
                                                in1=ps[:], op=ALU.add)

                linear_fm(o_bf, wo_d[i], F6, oproj_cb, w_tag="w")

                if i == 0:
                    tap("x1", x[:])
                # --- ffn ---
                h2 = actp.tile([128, F6, T], BF16, tag="h")
                layernorm(x, h2, 2 * i + 1)
                w1r = w1_d[i].ap().rearrange("(a p) n -> p a n", p=128)
                w2r = w2_d[i].ap().rearrange("(a p) n -> p a n", p=128)
                zs = []
                for j in range(FFN // 128):
                    wa = wpool.tile([128, F6, 128], BF16, tag="w")
                    nc.sync.dma_start(out=wa, in_=w1r[:, :, (2 * j) * 128:(2 * j + 1) * 128])
                    wg = wpool.tile([128, F6, 128], BF16, tag="w")
                    nc.sync.dma_start(out=wg, in_=w1r[:, :, (2 * j + 1) * 128:(2 * j + 2) * 128])
                    pa = psmm.tile([128, T], F32, tag="mm")
                    pg = psmm.tile([128, T], F32, tag="mm")
                    for k in range(F6):
                        nc.tensor.matmul(pa[:], wa[:, k, :], h2[:, k, :],
                                         start=(k == 0), stop=(k == F6 - 1))
                    for k in range(F6):
                        nc.tensor.matmul(pg[:], wg[:, k, :], h2[:, k, :],
                                         start=(k == 0), stop=(k == F6 - 1))
                    gg = smallp.tile([128, T], F32, tag="gg")
                    if has_f1b:
                        nc.vector.tensor_scalar_add(out=pa[:], in0=pa[:],
                                                    scalar1=f1b[:, 2 * j:2 * j + 1])
                        nc.scalar.activation(out=gg, in_=pg[:], func=AF.Gelu,
                                             bias=f1b[:, 2 * j + 1:2 * j + 2])
                    else:
                        nc.scalar.activation(out=gg, in_=pg[:], func=AF.Gelu)
                    z = zpool.tile([128, T], BF16, tag="z")
                    nc.vector.tensor_tensor(out=z, in0=pa[:], in1=gg, op=ALU.mult)
                    zs.append(z)
                if i == 0:
                    tap("z0", zs[0][:])
                for f in range(F6):
                    w2f = w2pool.tile([128, FFN // 128, 128], BF16, tag="w2")
                    nc.sync.dma_start(out=w2f, in_=w2r[:, :, f * 128:(f + 1) * 128])
                    ps = psmm.tile([128, T], F32, tag="mm")
                    for j in range(FFN // 128):
                        nc.tensor.matmul(ps[:], w2f[:, j, :], zs[j][:],
                                         start=(j == 0), stop=(j == FFN // 128 - 1))
                    if has_f2b:
                        tmp3 = smallp.tile([128, T], F32, tag="f2b_tmp")
                        nc.vector.tensor_scalar_add(out=tmp3, in0=ps[:],
                                                    scalar1=f2b[:, f:f + 1])
                        nc.vector.tensor_tensor(out=x[:, f, :], in0=x[:, f, :],
                                                in1=tmp3, op=ALU.add)
                    else:
                        nc.vector.tensor_tensor(out=x[:, f, :], in0=x[:, f, :],
                                                in1=ps[:], op=ALU.add)

            tap("x2", x[:])
            # ---------------- mlm head ----------------
            hf = actp.tile([128, F6, T], BF16, tag="h")
            layernorm(x, hf, 2 * NL)
            g1 = residp.tile([128, F6, T], F32)

            def mlm1_cb(m, ps, _):
                if has_m1b:
                    nc.scalar.activation(out=g1[:, m, :], in_=ps[:], func=AF.Gelu,
                                         bias=m1b[:, m:m + 1])
                else:
                    nc.scalar.activation(out=g1[:, m, :], in_=ps[:], func=AF.Gelu)

            linear_fm(hf, mw1_d, F6, mlm1_cb, w_tag="w")

            h2f = actp.tile([128, F6, T], BF16, tag="h")
            layernorm(g1, h2f, 2 * NL + 1)

            mw2r = mw2_d.ap().rearrange("(a p) n -> p a n", p=128)
            for vc in range(V // 512):
                wt = mw2pool.tile([128, F6, 512], BF16, tag="mw2")
                nc.sync.dma_start(out=wt, in_=mw2r[:, :, vc * 512:(vc + 1) * 512])
                for tt in range(TT):
                    ps = psh.tile([128, 512], F32, tag="mmh")
                    for k in range(F6):
                        nc.tensor.matmul(ps[:], h2f[:, k, tt * 128:(tt + 1) * 128],
                                         wt[:, k, :],
                                         start=(k == 0),
                                         stop=(k == F6 - 1 and not has_m2b))
                    if has_m2b:
                        m2bb = smallp.tile([1, 512], BF16, tag="m2bb")
                        nc.vector.tensor_copy(out=m2bb,
                                              in_=m2b[0:1, vc * 512:(vc + 1) * 512])
                        nc.tensor.matmul(ps[:], ones_row_bf[0:1, :],
                                         m2bb[0:1, :], start=False, stop=True)
                    osb = smallp.tile([128, 512], BF16, tag="osb")
                    nc.vector.tensor_copy(out=osb, in_=ps[:])
                    nc.sync.dma_start(
                        out=out_d.ap()[tt * 128:(tt + 1) * 128, vc * 512:(vc + 1) * 512],
                        in_=osb)

    nc.compile()
    _CACHE[key] = nc
    return nc


# ---------------------------------------------------------------------------
# host wrapper
# ---------------------------------------------------------------------------

def _host_pack(inputs):
    """Build the shared (core-independent) input arrays."""
    d = {}
    d["emb"] = np.ascontiguousarray(inputs["embed"], dtype=np.float32)
    d["ident"] = np.eye(128, dtype=np.float32)

    # alibi column-bias table [128, KT*H]: b[p, kt*H+h] = s_h*((kt*128+p)-(L-1)) + pad
    mask = np.asarray(inputs["attention_mask"]).reshape(L)
    pad = (mask == 0).astype(np.float32)           # reference adds +1.0 float mask
    j = np.arange(L, dtype=np.float32)
    # +ln(32) rescales the exp outputs into fp8e4m3's normal range; the
    # denominator row of v-hat scales identically so softmax cancels it.
    colb = (SLOPES[None, :] * (j[:, None] - (L - 1)) + pad[:, None]
            + math.log(32.0))                                        # [L, H]
    d["btab"] = np.ascontiguousarray(
        colb.reshape(KT, 128, H).transpose(1, 0, 2).reshape(128, KT * H)
    ).astype(np.float32)

    in_w = np.asarray(inputs["in_w"], dtype=np.float32)    # [NL, 3E, E]
    in_b = np.asarray(inputs["in_b"], dtype=np.float32)
    out_w = np.asarray(inputs["out_w"], dtype=np.float32)
    ffn_w1 = np.asarray(inputs["ffn_w1"], dtype=np.float32)
    ffn_w2 = np.asarray(inputs["ffn_w2"], dtype=np.float32)
    for i in range(NL):
        wqk = in_w[i, :2 * E].T.copy()                     # [E, 2E]
        d[f"wqk{i}"] = bf(wqk)
        d[f"wv{i}"] = bf(in_w[i, 2 * E:].T)                # [E, E] rhs layout
        d[f"wo{i}"] = bf(out_w[i].T)
        w1t = ffn_w1[i].T.reshape(E, 2, FFN // 128, 128)   # [E][a/g][j][128]
        w1t = w1t.transpose(0, 2, 1, 3).reshape(E, 2 * FFN)  # interleave a0 g0 a1 g1
        d[f"w1{i}"] = bf(w1t)
        d[f"w2{i}"] = bf(ffn_w2[i].T)                      # [FFN, E]
    d["mw1"] = bf(np.asarray(inputs["mlm_w1"], dtype=np.float32).T)
    d["mw2"] = bf(np.asarray(inputs["mlm_w2"], dtype=np.float32).T)   # [E, V]

    def pack_pf(vec):   # [E] -> [128, F6] feature-major per-partition
        return np.ascontiguousarray(
            np.asarray(vec, dtype=np.float32).reshape(F6, 128).T)

    lng, lnb = [], []
    for i in range(NL):
        lng.append(pack_pf(inputs["norm1_g"][i])); lnb.append(pack_pf(inputs["norm1_b"][i]))
        lng.append(pack_pf(inputs["ffn_g"][i])); lnb.append(pack_pf(inputs["ffn_bt"][i]))
    lng.append(pack_pf(inputs["fin_g"])); lnb.append(pack_pf(inputs["fin_b"]))
    lng.append(pack_pf(inputs["mlm_g"])); lnb.append(pack_pf(inputs["mlm_bt"]))
    d["lng"] = np.concatenate(lng, axis=1)
    d["lnb"] = np.concatenate(lnb, axis=1)

    qb = np.asarray(inputs["in_b"], dtype=np.float32)
    qkb = np.zeros((128, 2 * F6), np.float32)
    # note: per-layer biases differ; only support layer-invariant zero biases
    # in the fused path. If any nonzero, fall back handled via flags (we pack
    # layer 0's; correctness enforced by flag check in kernel()).
    qkvec = qb[0, :2 * E].copy()
    qkb[:, :] = qkvec.reshape(2 * F6, 128).T
    d["qkb"] = qkb
    d["vb"] = qb[0, 2 * E:].reshape(1, E).copy()
    d["ob"] = pack_pf(np.asarray(inputs["out_b"], dtype=np.float32)[0])
    f1 = np.asarray(inputs["ffn_b1"], dtype=np.float32)[0]
    f1r = f1.reshape(2, FFN // 128, 128).transpose(1, 0, 2).reshape(2 * FFN)
    d["f1b"] = np.ascontiguousarray(f1r.reshape(2 * FFN // 128, 128).T)
    d["f2b"] = pack_pf(np.asarray(inputs["ffn_b2"], dtype=np.float32)[0])
    d["m1b"] = pack_pf(np.asarray(inputs["mlm_b1"], dtype=np.float32))
    d["m2b"] = np.asarray(inputs["mlm_b2"], dtype=np.float32).reshape(1, V).copy()
    return d


def kernel(**inputs):
    shared = _host_pack(inputs)
    tokens = np.asarray(inputs["tokens"]).reshape(L)

    def nz(a):
        return bool(np.any(np.asarray(a) != 0))

    ln_affine = (nz(np.asarray(inputs["norm1_g"]) - 1) or nz(inputs["norm1_b"])
                 or nz(np.asarray(inputs["ffn_g"]) - 1) or nz(inputs["ffn_bt"])
                 or nz(np.asarray(inputs["fin_g"]) - 1) or nz(inputs["fin_b"])
                 or nz(np.asarray(inputs["mlm_g"]) - 1) or nz(inputs["mlm_bt"]))
    flags = (ln_affine,
             nz(inputs["in_b"][:, :2 * E]), nz(inputs["in_b"][:, 2 * E:]),
             nz(inputs["out_b"]), nz(inputs["ffn_b1"]), nz(inputs["ffn_b2"]),
             nz(inputs["mlm_b1"]), nz(inputs["mlm_b2"]))
    if any(flags[1:]) :
        # per-layer bias tensors packed only for layer 0; replicate properly
        # (all-zero in the reference problem, so this path is never hot)
        assert all(
            np.array_equal(np.asarray(inputs[k])[0], np.asarray(inputs[k])[j])
            for k in ("in_b", "out_b", "ffn_b1", "ffn_b2") for j in range(NL)
        ), "per-layer biases differing across layers not supported"

    nc = build(flags)

    in_maps = []
    for c in range(NC):
        m = dict(shared)
        m["tok"] = np.ascontiguousarray(
            tokens[c * T:(c + 1) * T].reshape(T, 1).astype(np.int32))
        in_maps.append(m)

    res = bass_utils.run_bass_kernel_spmd(
        nc, in_maps, core_ids=list(range(NC)), trace=TRACE)
    out = np.concatenate([res.results[c]["out"] for c in range(NC)], axis=0)
    kernel.last_result = res
    return out.astype(np.float32).reshape(B, L, V)



# revision 21
# speedup vs baseline: 1.1008x; 1.1008x over previous
"""DNABERT2 (4-layer BERT w/ ALiBi + GEGLU) forward pass on 8 Trainium2 cores.

Strategy: sequence-parallel over the 2048 tokens (256 tokens/core).
 - Residual stream x kept FEATURE-MAJOR in SBUF: [128 part, 6 ftile, 256 tok] fp32.
 - All matmul operands bf16 (weights cast on host; activations produced bf16).
 - Attention: scores computed TRANSPOSED ([key, query]) so the ALiBi column
   bias (slope_h * (j - (L-1)) + pad_j) is a per-partition ACT bias fused into
   the Exp op.  Softmax uses the fixed shift slope_h*(L-1) instead of a row
   max (shift-invariance; the q.k part is O(5) so exp cannot overflow).
 - PV matmul uses v-hat = [v | ones] per head (65 columns) so row 64 of the
   PV accumulator is the softmax denominator; normalization happens via
   reciprocal + PE ones-outer-product broadcast.
 - Per layer one packed AllGather ships each core's K/V shard (bf16) to all.
 - MLM head emits token-major [256, 4096] logits per core; host concatenates.
"""
import sys, math, os
sys.path.insert(0, "/opt/trn_rl_repo")

import numpy as np
import ml_dtypes

import concourse.bass as bass
import concourse.bacc as bacc
import concourse.tile as tile
from concourse import mybir
from concourse import bass_utils

AF = mybir.ActivationFunctionType
ALU = mybir.AluOpType
BF16 = mybir.dt.float16  # "half" dtype for matmul operands (fp16: 10-bit mantissa)
FP8 = mybir.dt.float8e4  # e4m3: K/V/attn-weight storage + AllGather payload
F32 = mybir.dt.float32
I32 = mybir.dt.int32

# model dims
V = 4096; E = 768; H = 12; NL = 4; FFN = 2048; B = 1; L = 2048; HD = 64
EPS = 1e-5
NC = 8            # cores
T = L // NC       # tokens per core = 256
TT = T // 128     # token tiles per core = 2
F6 = E // 128     # feature tiles = 6
KT = L // 128     # key tiles = 16
SCALE = 1.0 / math.sqrt(HD)

# config knobs
SKIP_THRESH = float(os.environ.get("KERN_SKIP_THRESH", "12"))  # 0 = no tile skipping
TRACE = os.environ.get("KERN_TRACE", "0") == "1"
NO_AG = os.environ.get("KERN_NO_AG", "0") == "1"  # timing-only: skip collective
DEBUG = os.environ.get("KERN_DEBUG", "0") == "1"


def _alibi_slopes(n):
    def pow2(m):
        start = 2.0 ** (-2.0 ** (-(math.log2(m) - 3)))
        return [start * start ** i for i in range(m)]
    if math.log2(n).is_integer():
        return np.array(pow2(n), dtype=np.float32)
    c = 2 ** math.floor(math.log2(n))
    s = pow2(c) + pow2(2 * c)[0::2][: n - c]
    return np.array(s, dtype=np.float32)

SLOPES = _alibi_slopes(H)  # (12,)


def kept_ktiles(h):
    """Key tiles whose max ALiBi bias is within SKIP_THRESH of the top;
    others underflow in the softmax and are skipped entirely."""
    if SKIP_THRESH <= 0:
        return list(range(KT))
    s = float(SLOPES[h])
    keep = [kt for kt in range(KT)
            if s * ((L - 1) - (kt * 128 + 127)) < SKIP_THRESH]
    return keep if keep else [KT - 1]


def bf(a):
    return np.ascontiguousarray(a).astype(np.float16)


# ---------------------------------------------------------------------------
# device program
# ---------------------------------------------------------------------------

_CACHE = {}


def build(flags, reps=1):
    key = (flags, reps)
    if key in _CACHE:
        return _CACHE[key]
    (ln_affine, has_qkb, has_vb, has_ob, has_f1b, has_f2b, has_m1b,
     has_m2b) = flags

    nc = bacc.Bacc("TRN2", target_bir_lowering=False, debug=False,
                   num_devices=NC)

    # ---- dram tensors ----
    tok_d = nc.dram_tensor("tok", [T, 1], I32, kind="ExternalInput")
    emb_d = nc.dram_tensor("emb", [V, E], F32, kind="ExternalInput")
    id_d = nc.dram_tensor("ident", [128, 128], F32, kind="ExternalInput")
    btab_d = nc.dram_tensor("btab", [128, KT * H], F32, kind="ExternalInput")
    wqk_d = [nc.dram_tensor(f"wqk{i}", [E, 2 * E], BF16, kind="ExternalInput") for i in range(NL)]
    wv_d = [nc.dram_tensor(f"wv{i}", [E, E], BF16, kind="ExternalInput") for i in range(NL)]
    wo_d = [nc.dram_tensor(f"wo{i}", [E, E], BF16, kind="ExternalInput") for i in range(NL)]
    w1_d = [nc.dram_tensor(f"w1{i}", [E, 2 * FFN], BF16, kind="ExternalInput") for i in range(NL)]
    w2_d = [nc.dram_tensor(f"w2{i}", [FFN, E], BF16, kind="ExternalInput") for i in range(NL)]
    mw1_d = nc.dram_tensor("mw1", [E, E], BF16, kind="ExternalInput")
    mw2_d = nc.dram_tensor("mw2", [E, V], BF16, kind="ExternalInput")
    # ln params packed [128, 6] each, one tensor [128, n*6]
    n_ln = 2 * NL + 2
    lng_d = nc.dram_tensor("lng", [128, n_ln * F6], F32, kind="ExternalInput")
    lnb_d = nc.dram_tensor("lnb", [128, n_ln * F6], F32, kind="ExternalInput")
    # linear biases (always declared; host passes zeros; ops emitted per flag)
    qkb_d = nc.dram_tensor("qkb", [128, 2 * F6], F32, kind="ExternalInput")
    vb_d = nc.dram_tensor("vb", [1, E], F32, kind="ExternalInput")
    ob_d = nc.dram_tensor("ob", [128, F6], F32, kind="ExternalInput")
    f1b_d = nc.dram_tensor("f1b", [128, 2 * FFN // 128], F32, kind="ExternalInput")
    f2b_d = nc.dram_tensor("f2b", [128, F6], F32, kind="ExternalInput")
    m1b_d = nc.dram_tensor("m1b", [128, F6], F32, kind="ExternalInput")
    m2b_d = nc.dram_tensor("m2b", [1, V], F32, kind="ExternalInput")
    out_d = nc.dram_tensor("out", [T, V], BF16, kind="ExternalOutput")
    dbg_d = {}
    if DEBUG:
        for nm, shape in [("x0", [128, F6 * T]), ("h0", [128, F6 * T]),
                          ("q0", [128, F6 * T]), ("k0", [128, F6 * T]),
                          ("v0", [128, TT * E]), ("ksb", [128, F6 * 128]),
                          ("vhb", [128, 2 * 780]), ("at0", [128, 2 * T]),
                          ("o0", [128, F6 * T]), ("x1", [128, F6 * T]),
                          ("sums", [H, T]), ("rrs", [H, T]),
                          ("ath", [128, H * T]), ("sch", [128, H * T]),
                          ("or1", [65, T]), ("bc1", [64, T]),
                          ("z0", [128, T]), ("x2", [128, F6 * T])]:
            dbg_d[nm] = nc.dram_tensor("dbg_" + nm, shape, F32,
                                       kind="ExternalOutput")

    KELEM = 128 * F6 * T          # k shard elems per core (196608)
    VELEM = T * E                 # v shard elems per core (196608)
    AGE = KELEM + VELEM           # merged K+V AllGather payload (fp16)

    from contextlib import ExitStack
    with tile.TileContext(nc) as tc, ExitStack() as _es:
        constp = _es.enter_context(tc.tile_pool(name="const", bufs=1))
        residp = _es.enter_context(tc.tile_pool(name="resid", bufs=1))
        actp = _es.enter_context(tc.tile_pool(name="act", bufs=2))
        kvp = _es.enter_context(tc.tile_pool(name="kv", bufs=1))
        attnp = _es.enter_context(tc.tile_pool(name="attn", bufs=1))
        smallp = _es.enter_context(tc.tile_pool(name="small", bufs=2))
        wpool = _es.enter_context(tc.tile_pool(name="wpool", bufs=6))
        w2pool = _es.enter_context(tc.tile_pool(name="w2pool", bufs=2))
        mw2pool = _es.enter_context(tc.tile_pool(name="mw2pool", bufs=1))
        zpool = _es.enter_context(tc.tile_pool(name="zpool", bufs=16))
        psmm = _es.enter_context(tc.tile_pool(name="psmm", bufs=2, space="PSUM"))
        pssc = _es.enter_context(tc.tile_pool(name="pssc", bufs=2, space="PSUM"))
        pspv = _es.enter_context(tc.tile_pool(name="pspv", bufs=2, space="PSUM"))
        psh = _es.enter_context(tc.tile_pool(name="psh", bufs=2, space="PSUM"))
        dramp = _es.enter_context(tc.tile_pool(name="dram", bufs=2, space="DRAM"))

        # ---- constants ----
        ident = constp.tile([128, 128], F32)
        nc.sync.dma_start(out=ident, in_=id_d.ap())
        btab = constp.tile([128, KT * H], F32)
        nc.sync.dma_start(out=btab, in_=btab_d.ap())
        inv_e_bf = constp.tile([128, 1], BF16)
        nc.vector.memset(inv_e_bf, 1.0 / E)
        ones_f = constp.tile([128, 128], F32)
        nc.vector.memset(ones_f, 1.0)
        ones_row_bf = constp.tile([1, 128], BF16)
        nc.vector.memset(ones_row_bf, 1.0)
        eps_t = constp.tile([128, 1], F32)
        nc.vector.memset(eps_t, EPS)
        lng = constp.tile([128, n_ln * F6], F32)
        nc.sync.dma_start(out=lng, in_=lng_d.ap())
        lnb = constp.tile([128, n_ln * F6], F32)
        nc.sync.dma_start(out=lnb, in_=lnb_d.ap())
        qkb = constp.tile([128, 2 * F6], F32)
        nc.sync.dma_start(out=qkb, in_=qkb_d.ap())
        vb = constp.tile([1, E], F32)
        nc.sync.dma_start(out=vb, in_=vb_d.ap())
        ob = constp.tile([128, F6], F32)
        nc.sync.dma_start(out=ob, in_=ob_d.ap())
        f1b = constp.tile([128, 2 * FFN // 128], F32)
        nc.sync.dma_start(out=f1b, in_=f1b_d.ap())
        f2b = constp.tile([128, F6], F32)
        nc.sync.dma_start(out=f2b, in_=f2b_d.ap())
        m1b = constp.tile([128, F6], F32)
        nc.sync.dma_start(out=m1b, in_=m1b_d.ap())
        m2b = constp.tile([1, V], F32)
        nc.sync.dma_start(out=m2b, in_=m2b_d.ap())

        def tap(nm, ap):
            if DEBUG and nm in dbg_d:
                shp = list(ap.shape)
                mid = int(np.prod(shp[1:-1])) if len(shp) > 2 else 1
                pad = shp[:-1] + [max(shp[-1], -(-(F6 * T) // mid))]
                t = smallp.tile(shp, F32, tag="dbgscratch", bufs=1,
                                padded_shape=pad)
                nc.vector.tensor_copy(out=t, in_=ap)
                nc.sync.dma_start(out=dbg_d[nm].ap().rearrange(
                    "p (a b) -> p a b", a=ap.shape[1]) if len(ap.shape) == 3
                    else dbg_d[nm].ap(), in_=t)

        # ---- residual stream ----
        x = residp.tile([128, F6, T], F32)

        for _rep in range(reps):
            # ---- embedding gather + transpose to feature-major ----
            tokt = constp.tile([128, TT], I32)
            nc.sync.dma_start(out=tokt,
                              in_=tok_d.ap().rearrange("(a p) o -> p (a o)", p=128))
            for tt in range(TT):
                xtm = actp.tile([128, E], F32, tag="xtm", bufs=1)
                nc.gpsimd.indirect_dma_start(
                    out=xtm[:], out_offset=None, in_=emb_d.ap(),
                    in_offset=bass.IndirectOffsetOnAxis(ap=tokt[:, tt:tt + 1], axis=0))
                for f in range(F6):
                    tp = psmm.tile([128, 128], F32, tag="mm")
                    nc.tensor.transpose(out=tp[:], in_=xtm[:, f * 128:(f + 1) * 128],
                                        identity=ident[:])
                    nc.vector.tensor_copy(out=x[:, f, tt * 128:(tt + 1) * 128],
                                          in_=tp[:])

            # ---------------- helpers ----------------
            def layernorm(src, h_dst, ln_idx):
                """src fp32 [128,F6,T] -> h_dst bf16 [128,F6,T] (LN over features)."""
                g_ap = lng[:, ln_idx * F6:(ln_idx + 1) * F6]
                b_ap = lnb[:, ln_idx * F6:(ln_idx + 1) * F6]
                # cat holds [x | x^2] side by side so one matmul per ftile
                # produces both sums; inv_e lhsT folds the 1/E scaling.
                cat = kvp.tile([128, F6, 2, T], BF16, tag="ln_xb")
                nc.vector.tensor_copy(out=cat[:, :, 0, :], in_=src)
                nc.vector.tensor_tensor(out=cat[:, :, 1, :], in0=cat[:, :, 0, :],
                                        in1=cat[:, :, 0, :], op=ALU.mult)
                psAB = pssc.tile([1, 2 * T], F32, tag="sc")
                for k in range(F6):
                    nc.tensor.matmul(psAB[:], inv_e_bf[:, 0:1], cat[:, k, :, :],
                                     start=(k == 0), stop=(k == F6 - 1))
                st = smallp.tile([1, 4 * T], F32, tag="ln_st", bufs=1)
                # slots: 0 m, 1 ms/var/sqrt, 2 rstd, 3 mrstd  (2,3 adjacent for bcast)
                sl = lambda i: st[:, i * T:(i + 1) * T]
                nc.vector.tensor_copy(out=st[:, 0:2 * T], in_=psAB[:])
                nc.vector.tensor_tensor(out=sl(3), in0=sl(0), in1=sl(0), op=ALU.mult)
                nc.vector.tensor_tensor(out=sl(1), in0=sl(1), in1=sl(3),
                                        op=ALU.subtract)          # var
                nc.scalar.activation(out=sl(1), in_=sl(1), func=AF.Sqrt,
                                     bias=eps_t[0:1, 0:1])
                nc.vector.reciprocal_approx_fast(out=sl(2), in_=sl(1))   # rstd
                nc.vector.tensor_tensor(out=sl(3), in0=sl(0), in1=sl(2), op=ALU.mult)
                bc = pssc.tile([128, 2 * T], F32, tag="sc")
                nc.tensor.matmul(bc[:], ones_f[0:1, 0:128], st[0:1, 2 * T:4 * T],
                                 start=True, stop=True)
                for f in range(F6):
                    tmp = smallp.tile([128, T], F32, tag="ln_tmp")
                    nc.vector.tensor_tensor(out=tmp, in0=src[:, f, :],
                                            in1=bc[:, 0:T], op=ALU.mult)
                    nc.vector.tensor_tensor(out=h_dst[:, f, :], in0=tmp,
                                            in1=bc[:, T:2 * T], op=ALU.subtract)
                    if ln_affine:
                        nc.vector.tensor_scalar(
                            out=h_dst[:, f, :], in0=h_dst[:, f, :],
                            scalar1=g_ap[:, f:f + 1], scalar2=b_ap[:, f:f + 1],
                            op0=ALU.mult, op1=ALU.add)

            def linear_fm(h_bf, w_dram, mtiles, out_cb, bias_tile=None,
                          w_tag="w"):
                """out[m] = W.T-tile-m @ h (feature-major out), psum -> out_cb."""
                for m in range(mtiles):
                    wt = wpool.tile([128, F6, 128], BF16, tag=w_tag)
                    nc.sync.dma_start(
                        out=wt,
                        in_=w_dram.ap().rearrange("(a p) n -> p a n", p=128)[
                            :, :, m * 128:(m + 1) * 128])
                    ps = psmm.tile([128, T], F32, tag="mm")
                    for k in range(F6):
                        nc.tensor.matmul(ps[:], wt[:, k, :], h_bf[:, k, :],
                                         start=(k == 0), stop=(k == F6 - 1))
                    out_cb(m, ps, bias_tile)

            tap("x0", x[:])
            # ---------------- layers ----------------
            for i in range(NL):
                h_bf = actp.tile([128, F6, T], BF16, tag="h")
                layernorm(x, h_bf, 2 * i)
                if i == 0:
                    tap("h0", h_bf[:])

                # --- qkv: K first so its AllGather launches ASAP ---
                q_bf = actp.tile([128, F6, T], BF16, tag="q", bufs=1)
                kshard = actp.tile([128, F6, T], BF16, tag="kshard", bufs=1)
                wqkr = wqk_d[i].ap().rearrange("(a p) n -> p a n", p=128)

                def qk_mm(m, dst):
                    wt = wpool.tile([128, F6, 128], BF16, tag="w")
                    nc.sync.dma_start(out=wt,
                                      in_=wqkr[:, :, m * 128:(m + 1) * 128])
                    ps = psmm.tile([128, T], F32, tag="mm")
                    for k in range(F6):
                        nc.tensor.matmul(ps[:], wt[:, k, :], h_bf[:, k, :],
                                         start=(k == 0), stop=(k == F6 - 1))
                    if has_qkb:
                        nc.vector.tensor_scalar_add(out=dst, in0=ps[:],
                                                    scalar1=qkb[:, m:m + 1])
                    else:
                        nc.vector.tensor_copy(out=dst, in_=ps[:])

                for m in range(F6, 2 * F6):
                    qk_mm(m, kshard[:, m - F6, :])

                ag_in = dramp.tile([1, AGE], BF16, tag="ag_in")
                ag_out = dramp.tile([NC, AGE], BF16, tag="ag_out",
                                    addr_space="Shared")
                nc.sync.dma_start(
                    out=ag_in[0:1, 0:KELEM].rearrange("o (p f t) -> (o p) f t",
                                                      p=128, f=F6),
                    in_=kshard)

                # --- V projection (packed into the same AllGather as K) ---
                vshard = actp.tile([128, TT, E], BF16, tag="vshard", bufs=1)
                wvt = kvp.tile([128, F6, E], BF16, tag="wv")
                nc.sync.dma_start(out=wvt,
                                  in_=wv_d[i].ap().rearrange("(a p) n -> p a n", p=128))
                for tt in range(TT):
                    for n0 in (0, 384):
                        ps = psmm.tile([128, 384], F32, tag="mm")
                        for k in range(F6):
                            nc.tensor.matmul(ps[:], h_bf[:, k, tt * 128:(tt + 1) * 128],
                                             wvt[:, k, n0:n0 + 384],
                                             start=(k == 0), stop=(k == F6 - 1))
                        if has_vb:
                            bcv = pssc.tile([128, 384], F32, tag="sc")
                            nc.tensor.matmul(bcv[:], ones_f[0:1, :],
                                             vb[0:1, n0:n0 + 384], start=True, stop=True)
                            nc.vector.tensor_tensor(out=vshard[:, tt, n0:n0 + 384],
                                                    in0=ps[:], in1=bcv[:], op=ALU.add)
                        else:
                            nc.vector.tensor_copy(out=vshard[:, tt, n0:n0 + 384],
                                                  in_=ps[:])
                if i == 0:
                    tap("v0", vshard[:])
                nc.sync.dma_start(
                    out=ag_in[0:1, KELEM:AGE].rearrange(
                        "o (tt p e) -> (o p) tt e", tt=TT, p=128),
                    in_=vshard)
                if NO_AG:
                    nc.sync.dma_start(out=ag_out[0:1, :], in_=ag_in[:])
                else:
                    nc.gpsimd.collective_compute(
                        "AllGather", ALU.bypass,
                        replica_groups=[list(range(NC))],
                        ins=[ag_in[:]], outs=[ag_out[:]])
                k_sb = kvp.tile([128, F6, L], BF16, tag="k_sb")
                # load only the key-tile tail ranges any head of this
                # feature-pair keeps (ALiBi tile skipping)
                ksrc = ag_out[:, 0:KELEM].rearrange("r (p f t) -> p f r t",
                                                    p=128, f=F6)
                kdst = k_sb.rearrange("p f (r t) -> p f r t", r=NC)
                for f0 in range(F6):
                    kmin = min(kept_ktiles(2 * f0)[0], kept_ktiles(2 * f0 + 1)[0])
                    r0 = kmin // TT
                    nc.sync.dma_start(out=kdst[:, f0:f0 + 1, r0:NC, :],
                                      in_=ksrc[:, f0:f0 + 1, r0:NC, :])
                vhat = kvp.tile([128, KT, H * 65], BF16, tag="vhat")
                VROW = H * 65  # 780 elems per (partition, ktile)
                for kt in range(KT):
                    r, tt = kt // TT, kt % TT
                    off = tt * (128 * E)
                    # heads that keep this key tile, as contiguous runs
                    hs = [h for h in range(H) if kept_ktiles(h)[0] <= kt]
                    runs = []
                    for h in hs:
                        if runs and runs[-1][1] == h - 1:
                            runs[-1][1] = h
                        else:
                            runs.append([h, h])
                    vsrc = ag_out[r:r + 1, KELEM + off:KELEM + off + 128 * E
                                  ].rearrange("r (p h d) -> (r p) h d", p=128, h=H)
                    for h0, h1 in runs:
                        nh = h1 - h0 + 1
                        nc.sync.dma_start(
                            out=bass.AP(tensor=vhat.tensor,
                                        offset=vhat.offset + kt * VROW + h0 * 65,
                                        ap=[vhat.ap[0],
                                            [65, nh],           # h
                                            [1, 64]]),          # d
                            in_=vsrc[:, h0:h1 + 1, :])
                # ones columns (col 64 of each head block)
                nc.vector.memset(
                    bass.AP(tensor=vhat.tensor, offset=vhat.offset + 64,
                            ap=[vhat.ap[0], [VROW, KT], [65, H]]),
                    1.0)

                # --- Q projection (overlaps the AllGathers) ---
                for m in range(F6):
                    qk_mm(m, q_bf[:, m, :])
                if i == 0:
                    tap("q0", q_bf[:])
                    tap("k0", kshard[:])
                if i == 0:
                    tap("ksb", k_sb[:, :, 0:128])
                    tap("vhb", vhat[:, 0:2, :])
                # --- attention: QK+exp for ALL heads first (overlaps the V
                # AllGather), then all PV+normalize ---
                o_bf = actp.tile([128, F6, T], BF16, tag="o", bufs=1)
                attnTs = []
                for h in range(H):
                    f0, r0 = h // 2, (h % 2) * 64
                    kts = kept_ktiles(h)
                    attnT = attnp.tile([128, len(kts), T], BF16,
                                       tag=f"attnT{h}", name=f"attnT{h}")
                    for g0 in range(0, len(kts), 2):
                        grp = kts[g0:g0 + 2]
                        sc = pssc.tile([128, len(grp) * T], F32, tag="sc")
                        for j, kt in enumerate(grp):
                            nc.tensor.matmul(
                                sc[:, j * T:(j + 1) * T],
                                k_sb[r0:r0 + 64, f0, kt * 128:(kt + 1) * 128],
                                q_bf[r0:r0 + 64, f0, :],
                                start=True, stop=True)
                            if DEBUG and i == 0 and kt == KT - 1:
                                scht = smallp.tile([128, T], F32, tag="scht")
                                nc.vector.tensor_copy(out=scht,
                                                      in_=sc[:, j * T:(j + 1) * T])
                                nc.sync.dma_start(
                                    out=dbg_d["sch"].ap()[:, h * T:(h + 1) * T],
                                    in_=scht)
                            nc.scalar.activation(
                                out=attnT[:, g0 + j, :], in_=sc[:, j * T:(j + 1) * T],
                                func=AF.Exp, bias=btab[:, kt * H + h:kt * H + h + 1],
                                scale=SCALE)
                    if DEBUG and i == 0:
                        atht = smallp.tile([128, T], F32, tag="atht")
                        nc.vector.tensor_copy(out=atht, in_=attnT[:, len(kts) - 1, :])
                        nc.sync.dma_start(out=dbg_d["ath"].ap()[:, h * T:(h + 1) * T],
                                          in_=atht)
                    attnTs.append((attnT, kts))
                for h in range(H):
                    f0, r0 = h // 2, (h % 2) * 64
                    attnT, kts = attnTs[h]
                    if i == 0 and h == 0 and len(kts) >= 2 and kts[0] == 0:
                        tap("at0", attnT[:, 0:2, :])
                    pv = pspv.tile([65, T], F32, tag="pv")
                    for n, kt in enumerate(kts):
                        nc.tensor.matmul(pv[:], vhat[:, kt, h * 65:h * 65 + 65],
                                         attnT[:, n, :],
                                         start=(n == 0), stop=(n == len(kts) - 1))
                    o_raw = smallp.tile([65, T], F32, tag="o_raw")
                    nc.scalar.activation(out=o_raw, in_=pv[:], func=AF.Copy)
                    srow = smallp.tile([1, T], F32, tag="srow")
                    nc.vector.tensor_copy(out=srow[0:1, :], in_=o_raw[64:65, :])
                    rr = smallp.tile([1, T], F32, tag="rr")
                    nc.vector.reciprocal_approx_fast(out=rr[0:1, :],
                                                     in_=srow[0:1, :])
                    bc = psmm.tile([64, T], F32, tag="mm")
                    nc.tensor.matmul(bc[:], ones_f[0:1, 0:64], rr[0:1, :],
                                     start=True, stop=True)
                    if DEBUG and i == 0:
                        nc.sync.dma_start(out=dbg_d["sums"].ap()[h:h + 1, :],
                                          in_=o_raw[64:65, :])
                        nc.sync.dma_start(out=dbg_d["rrs"].ap()[h:h + 1, :],
                                          in_=rr[0:1, :])
                        if h == 1:
                            nc.sync.dma_start(out=dbg_d["or1"].ap(), in_=o_raw[:])
                            bc1t = smallp.tile([64, T], F32, tag="bc1t", bufs=1)
                            nc.vector.tensor_copy(out=bc1t, in_=bc[:])
                            nc.sync.dma_start(out=dbg_d["bc1"].ap(), in_=bc1t)
                    nc.vector.tensor_tensor(out=o_bf[r0:r0 + 64, f0, :],
                                            in0=o_raw[0:64, :], in1=bc[:],
                                            op=ALU.mult)
                # --- out_proj + residual ---
                def oproj_cb(m, ps, _):
                    if has_ob:
                        tmp2 = smallp.tile([128, T], F32, tag="ob_tmp")
                        nc.vector.tensor_scalar_add(out=tmp2, in0=ps[:],
                                                    scalar1=ob[:, m:m + 1])
                        nc.vector.tensor_tensor(out=x[:, m, :], in0=x[:, m, :],
                                                in1=tmp2, op=ALU.add)
                    else:
                        nc.vector.tensor_tensor(out=x[:, m, :], in0=x[:, m, :],
                                                in1=ps[:], op=ALU.add)

                linear_fm(o_bf, wo_d[i], F6, oproj_cb, w_tag="w")

                if i == 0:
                    tap("x1", x[:])
                # --- ffn ---
                h2 = actp.tile([128, F6, T], BF16, tag="h")
                layernorm(x, h2, 2 * i + 1)
                w1r = w1_d[i].ap().rearrange("(a p) n -> p a n", p=128)
                w2r = w2_d[i].ap().rearrange("(a p) n -> p a n", p=128)
                zs = []
                for j in range(FFN // 128):
                    wa = wpool.tile([128, F6, 128], BF16, tag="w")
                    nc.sync.dma_start(out=wa, in_=w1r[:, :, (2 * j) * 128:(2 * j + 1) * 128])
                    wg = wpool.tile([128, F6, 128], BF16, tag="w")
                    nc.sync.dma_start(out=wg, in_=w1r[:, :, (2 * j + 1) * 128:(2 * j + 2) * 128])
                    pa = psmm.tile([128, T], F32, tag="mm")
                    pg = psmm.tile([128, T], F32, tag="mm")
                    for k in range(F6):
                        nc.tensor.matmul(pa[:], wa[:, k, :], h2[:, k, :],
                                         start=(k == 0), stop=(k == F6 - 1))
                    for k in range(F6):
                        nc.tensor.matmul(pg[:], wg[:, k, :], h2[:, k, :],
                                         start=(k == 0), stop=(k == F6 - 1))
                    gg = smallp.tile([128, T], F32, tag="gg")
                    if has_f1b:
                        nc.vector.tensor_scalar_add(out=pa[:], in0=pa[:],
                                                    scalar1=f1b[:, 2 * j:2 * j + 1])
                        nc.scalar.activation(out=gg, in_=pg[:], func=AF.Gelu,
                                             bias=f1b[:, 2 * j + 1:2 * j + 2])
                    else:
                        nc.scalar.activation(out=gg, in_=pg[:], func=AF.Gelu)
                    z = zpool.tile([128, T], BF16, tag="z")
                    nc.vector.tensor_tensor(out=z, in0=pa[:], in1=gg, op=ALU.mult)
                    zs.append(z)
                if i == 0:
                    tap("z0", zs[0][:])
                for f in range(F6):
                    w2f = w2pool.tile([128, FFN // 128, 128], BF16, tag="w2")
                    nc.sync.dma_start(out=w2f, in_=w2r[:, :, f * 128:(f + 1) * 128])
                    ps = psmm.tile([128, T], F32, tag="mm")
                    for j in range(FFN // 128):
                        nc.tensor.matmul(ps[:], w2f[:, j, :], zs[j][:],
                                         start=(j == 0), stop=(j == FFN // 128 - 1))
                    if has_f2b:
                        tmp3 = smallp.tile([128, T], F32, tag="f2b_tmp")
                        nc.vector.tensor_scalar_add(out=tmp3, in0=ps[:],
                                                    scalar1=f2b[:, f:f + 1])
                        nc.vector.tensor_tensor(out=x[:, f, :], in0=x[:, f, :],
                                                in1=tmp3, op=ALU.add)
                    else:
                        nc.vector.tensor_tensor(out=x[:, f, :], in0=x[:, f, :],
                                                in1=ps[:], op=ALU.add)

            tap("x2", x[:])
            # ---------------- mlm head ----------------
            hf = actp.tile([128, F6, T], BF16, tag="h")
            layernorm(x, hf, 2 * NL)
            g1 = residp.tile([128, F6, T], F32)

            def mlm1_cb(m, ps, _):
                if has_m1b:
                    nc.scalar.activation(out=g1[:, m, :], in_=ps[:], func=AF.Gelu,
                                         bias=m1b[:, m:m + 1])
                else:
                    nc.scalar.activation(out=g1[:, m, :], in_=ps[:], func=AF.Gelu)

            linear_fm(hf, mw1_d, F6, mlm1_cb, w_tag="w")

            h2f = actp.tile([128, F6, T], BF16, tag="h")
            layernorm(g1, h2f, 2 * NL + 1)

            mw2r = mw2_d.ap().rearrange("(a p) n -> p a n", p=128)
            for vc in range(V // 512):
                wt = mw2pool.tile([128, F6, 512], BF16, tag="mw2")
                nc.sync.dma_start(out=wt, in_=mw2r[:, :, vc * 512:(vc + 1) * 512])
                for tt in range(TT):
                    ps = psh.tile([128, 512], F32, tag="mmh")
                    for k in range(F6):
                        nc.tensor.matmul(ps[:], h2f[:, k, tt * 128:(tt + 1) * 128],
                                         wt[:, k, :],
                                         start=(k == 0),
                                         stop=(k == F6 - 1 and not has_m2b))
                    if has_m2b:
                        m2bb = smallp.tile([1, 512], BF16, tag="m2bb")
                        nc.vector.tensor_copy(out=m2bb,
                                              in_=m2b[0:1, vc * 512:(vc + 1) * 512])
                        nc.tensor.matmul(ps[:], ones_row_bf[0:1, :],
                                         m2bb[0:1, :], start=False, stop=True)
                    osb = smallp.tile([128, 512], BF16, tag="osb")
                    nc.vector.tensor_copy(out=osb, in_=ps[:])
                    nc.sync.dma_start(
                        out=out_d.ap()[tt * 128:(tt + 1) * 128, vc * 512:(vc + 1) * 512],
                        in_=osb)

    nc.compile()
    _CACHE[key] = nc
    return nc


# ---------------------------------------------------------------------------
# host wrapper
# ---------------------------------------------------------------------------

def _host_pack(inputs):
    """Build the shared (core-independent) input arrays."""
    d = {}
    d["emb"] = np.ascontiguousarray(inputs["embed"], dtype=np.float32)
    d["ident"] = np.eye(128, dtype=np.float32)

    # alibi column-bias table [128, KT*H]: b[p, kt*H+h] = s_h*((kt*128+p)-(L-1)) + pad
    mask = np.asarray(inputs["attention_mask"]).reshape(L)
    pad = (mask == 0).astype(np.float32)           # reference adds +1.0 float mask
    j = np.arange(L, dtype=np.float32)
    # +ln(32) rescales the exp outputs into fp8e4m3's normal range; the
    # denominator row of v-hat scales identically so softmax cancels it.
    colb = (SLOPES[None, :] * (j[:, None] - (L - 1)) + pad[:, None]
            + math.log(32.0))                                        # [L, H]
    d["btab"] = np.ascontiguousarray(
        colb.reshape(KT, 128, H).transpose(1, 0, 2).reshape(128, KT * H)
    ).astype(np.float32)

    in_w = np.asarray(inputs["in_w"], dtype=np.float32)    # [NL, 3E, E]
    in_b = np.asarray(inputs["in_b"], dtype=np.float32)
    out_w = np.asarray(inputs["out_w"], dtype=np.float32)
    ffn_w1 = np.asarray(inputs["ffn_w1"], dtype=np.float32)
    ffn_w2 = np.asarray(inputs["ffn_w2"], dtype=np.float32)
    for i in range(NL):
        wqk = in_w[i, :2 * E].T.copy()                     # [E, 2E]
        d[f"wqk{i}"] = bf(wqk)
        d[f"wv{i}"] = bf(in_w[i, 2 * E:].T)                # [E, E] rhs layout
        d[f"wo{i}"] = bf(out_w[i].T)
        w1t = ffn_w1[i].T.reshape(E, 2, FFN // 128, 128)   # [E][a/g][j][128]
        w1t = w1t.transpose(0, 2, 1, 3).reshape(E, 2 * FFN)  # interleave a0 g0 a1 g1
        d[f"w1{i}"] = bf(w1t)
        d[f"w2{i}"] = bf(ffn_w2[i].T)                      # [FFN, E]
    d["mw1"] = bf(np.asarray(inputs["mlm_w1"], dtype=np.float32).T)
    d["mw2"] = bf(np.asarray(inputs["mlm_w2"], dtype=np.float32).T)   # [E, V]

    def pack_pf(vec):   # [E] -> [128, F6] feature-major per-partition
        return np.ascontiguousarray(
            np.asarray(vec, dtype=np.float32).reshape(F6, 128).T)

    lng, lnb = [], []
    for i in range(NL):
        lng.append(pack_pf(inputs["norm1_g"][i])); lnb.append(pack_pf(inputs["norm1_b"][i]))
        lng.append(pack_pf(inputs["ffn_g"][i])); lnb.append(pack_pf(inputs["ffn_bt"][i]))
    lng.append(pack_pf(inputs["fin_g"])); lnb.append(pack_pf(inputs["fin_b"]))
    lng.append(pack_pf(inputs["mlm_g"])); lnb.append(pack_pf(inputs["mlm_bt"]))
    d["lng"] = np.concatenate(lng, axis=1)
    d["lnb"] = np.concatenate(lnb, axis=1)

    qb = np.asarray(inputs["in_b"], dtype=np.float32)
    qkb = np.zeros((128, 2 * F6), np.float32)
    # note: per-layer biases differ; only support layer-invariant zero biases
    # in the fused path. If any nonzero, fall back handled via flags (we pack
    # layer 0's; correctness enforced by flag check in kernel()).
    qkvec = qb[0, :2 * E].copy()
    qkb[:, :] = qkvec.reshape(2 * F6, 128).T
    d["qkb"] = qkb
    d["vb"] = qb[0, 2 * E:].reshape(1, E).copy()
    d["ob"] = pack_pf(np.asarray(inputs["out_b"], dtype=np.float32)[0])
    f1 = np.asarray(inputs["ffn_b1"], dtype=np.float32)[0]
    f1r = f1.reshape(2, FFN // 128, 128).transpose(1, 0, 2).reshape(2 * FFN)
    d["f1b"] = np.ascontiguousarray(f1r.reshape(2 * FFN // 128, 128).T)
    d["f2b"] = pack_pf(np.asarray(inputs["ffn_b2"], dtype=np.float32)[0])
    d["m1b"] = pack_pf(np.asarray(inputs["mlm_b1"], dtype=np.float32))
    d["m2b"] = np.asarray(inputs["mlm_b2"], dtype=np.float32).reshape(1, V).copy()
    return d


def kernel(**inputs):
    shared = _host_pack(inputs)
    tokens = np.asarray(inputs["tokens"]).reshape(L)

    def nz(a):
        return bool(np.any(np.asarray(a) != 0))

    ln_affine = (nz(np.asarray(inputs["norm1_g"]) - 1) or nz(inputs["norm1_b"])
                 or nz(np.asarray(inputs["ffn_g"]) - 1) or nz(inputs["ffn_bt"])
                 or nz(np.asarray(inputs["fin_g"]) - 1) or nz(inputs["fin_b"])
                 or nz(np.asarray(inputs["mlm_g"]) - 1) or nz(inputs["mlm_bt"]))
    flags = (ln_affine,
             nz(inputs["in_b"][:, :2 * E]), nz(inputs["in_b"][:, 2 * E:]),
             nz(inputs["out_b"]), nz(inputs["ffn_b1"]), nz(inputs["ffn_b2"]),
             nz(inputs["mlm_b1"]), nz(inputs["mlm_b2"]))
    if any(flags[1:]) :
        # per-layer bias tensors packed only for layer 0; replicate properly
        # (all-zero in the reference problem, so this path is never hot)
        assert all(
            np.array_equal(np.asarray(inputs[k])[0], np.asarray(inputs[k])[j])
            for k in ("in_b", "out_b", "ffn_b1", "ffn_b2") for j in range(NL)
        ), "per-layer biases differing across layers not supported"

    nc = build(flags)

    in_maps = []
    for c in range(NC):
        m = dict(shared)
        m["tok"] = np.ascontiguousarray(
            tokens[c * T:(c + 1) * T].reshape(T, 1).astype(np.int32))
        in_maps.append(m)

    res = bass_utils.run_bass_kernel_spmd(
        nc, in_maps, core_ids=list(range(NC)), trace=TRACE)
    out = np.concatenate([res.results[c]["out"] for c in range(NC)], axis=0)
    kernel.last_result = res
    return out.astype(np.float32).reshape(B, L, V)



# revision 22
# speedup vs baseline: 1.5413x; 1.4001x over previous
"""DNABERT2 (4-layer BERT w/ ALiBi + GEGLU) forward pass on 8 Trainium2 cores.

Strategy: sequence-parallel over the 2048 tokens (256 tokens/core).
 - Residual stream x kept FEATURE-MAJOR in SBUF: [128 part, 6 ftile, 256 tok] fp32.
 - All matmul operands bf16 (weights cast on host; activations produced bf16).
 - Attention: scores computed TRANSPOSED ([key, query]) so the ALiBi column
   bias (slope_h * (j - (L-1)) + pad_j) is a per-partition ACT bias fused into
   the Exp op.  Softmax uses the fixed shift slope_h*(L-1) instead of a row
   max (shift-invariance; the q.k part is O(5) so exp cannot overflow).
 - PV matmul uses v-hat = [v | ones] per head (65 columns) so row 64 of the
   PV accumulator is the softmax denominator; normalization happens via
   reciprocal + PE ones-outer-product broadcast.
 - Per layer one packed AllGather ships each core's K/V shard (bf16) to all.
 - MLM head emits token-major [256, 4096] logits per core; host concatenates.
"""
import sys, math, os
sys.path.insert(0, "/opt/trn_rl_repo")

import numpy as np
import ml_dtypes

import concourse.bass as bass
import concourse.bacc as bacc
import concourse.tile as tile
from concourse import mybir
from concourse import bass_utils

AF = mybir.ActivationFunctionType
ALU = mybir.AluOpType
BF16 = mybir.dt.float16  # "half" dtype for matmul operands (fp16: 10-bit mantissa)
FP8 = mybir.dt.float8e4  # e4m3: K/V/attn-weight storage + AllGather payload
F32 = mybir.dt.float32
I32 = mybir.dt.int32

# model dims
V = 4096; E = 768; H = 12; NL = 4; FFN = 2048; B = 1; L = 2048; HD = 64
EPS = 1e-5
NC = 8            # cores
T = L // NC       # tokens per core = 256
TT = T // 128     # token tiles per core = 2
F6 = E // 128     # feature tiles = 6
KT = L // 128     # key tiles = 16
SCALE = 1.0 / math.sqrt(HD)

# config knobs
SKIP_THRESH = float(os.environ.get("KERN_SKIP_THRESH", "12"))  # 0 = no tile skipping
TRACE = os.environ.get("KERN_TRACE", "0") == "1"
NO_AG = os.environ.get("KERN_NO_AG", "0") == "1"  # timing-only: skip collective
DEBUG = os.environ.get("KERN_DEBUG", "0") == "1"


def _alibi_slopes(n):
    def pow2(m):
        start = 2.0 ** (-2.0 ** (-(math.log2(m) - 3)))
        return [start * start ** i for i in range(m)]
    if math.log2(n).is_integer():
        return np.array(pow2(n), dtype=np.float32)
    c = 2 ** math.floor(math.log2(n))
    s = pow2(c) + pow2(2 * c)[0::2][: n - c]
    return np.array(s, dtype=np.float32)

SLOPES = _alibi_slopes(H)  # (12,)


def kept_ktiles(h):
    """Key tiles whose max ALiBi bias is within SKIP_THRESH of the top;
    others underflow in the softmax and are skipped entirely."""
    if SKIP_THRESH <= 0:
        return list(range(KT))
    s = float(SLOPES[h])
    keep = [kt for kt in range(KT)
            if s * ((L - 1) - (kt * 128 + 127)) < SKIP_THRESH]
    return keep if keep else [KT - 1]


def bf(a):
    return np.ascontiguousarray(a).astype(np.float16)


# ---------------------------------------------------------------------------
# device program
# ---------------------------------------------------------------------------

_CACHE = {}


def build(flags, reps=1):
    key = (flags, reps)
    if key in _CACHE:
        return _CACHE[key]
    (ln_affine, has_qkb, has_vb, has_ob, has_f1b, has_f2b, has_m1b,
     has_m2b) = flags

    nc = bacc.Bacc("TRN2", target_bir_lowering=False, debug=False,
                   num_devices=NC)

    # ---- dram tensors ----
    tok_d = nc.dram_tensor("tok", [T, 1], I32, kind="ExternalInput")
    emb_d = nc.dram_tensor("emb", [V, E], F32, kind="ExternalInput")
    id_d = nc.dram_tensor("ident", [128, 128], F32, kind="ExternalInput")
    btab_d = nc.dram_tensor("btab", [128, KT * H], F32, kind="ExternalInput")
    wqk_d = [nc.dram_tensor(f"wqk{i}", [E, 2 * E], BF16, kind="ExternalInput") for i in range(NL)]
    wv_d = [nc.dram_tensor(f"wv{i}", [E, E], BF16, kind="ExternalInput") for i in range(NL)]
    wo_d = [nc.dram_tensor(f"wo{i}", [E, E], BF16, kind="ExternalInput") for i in range(NL)]
    w1_d = [nc.dram_tensor(f"w1{i}", [E, 2 * FFN], BF16, kind="ExternalInput") for i in range(NL)]
    w2_d = [nc.dram_tensor(f"w2{i}", [FFN, E], BF16, kind="ExternalInput") for i in range(NL)]
    mw1_d = nc.dram_tensor("mw1", [E, E], BF16, kind="ExternalInput")
    mw2_d = nc.dram_tensor("mw2", [E, V], BF16, kind="ExternalInput")
    # ln params packed [128, 6] each, one tensor [128, n*6]
    n_ln = 2 * NL + 2
    lng_d = nc.dram_tensor("lng", [128, n_ln * F6], F32, kind="ExternalInput")
    lnb_d = nc.dram_tensor("lnb", [128, n_ln * F6], F32, kind="ExternalInput")
    # linear biases (always declared; host passes zeros; ops emitted per flag)
    qkb_d = nc.dram_tensor("qkb", [128, 2 * F6], F32, kind="ExternalInput")
    vb_d = nc.dram_tensor("vb", [1, E], F32, kind="ExternalInput")
    ob_d = nc.dram_tensor("ob", [128, F6], F32, kind="ExternalInput")
    f1b_d = nc.dram_tensor("f1b", [128, 2 * FFN // 128], F32, kind="ExternalInput")
    f2b_d = nc.dram_tensor("f2b", [128, F6], F32, kind="ExternalInput")
    m1b_d = nc.dram_tensor("m1b", [128, F6], F32, kind="ExternalInput")
    m2b_d = nc.dram_tensor("m2b", [1, V], F32, kind="ExternalInput")
    out_d = nc.dram_tensor("out", [T, V], BF16, kind="ExternalOutput")
    dbg_d = {}
    if DEBUG:
        for nm, shape in [("x0", [128, F6 * T]), ("h0", [128, F6 * T]),
                          ("q0", [128, F6 * T]), ("k0", [128, F6 * T]),
                          ("v0", [128, TT * E]), ("ksb", [128, F6 * 128]),
                          ("vhb", [128, 2 * 780]), ("at0", [128, 2 * T]),
                          ("o0", [128, F6 * T]), ("x1", [128, F6 * T]),
                          ("sums", [H, T]), ("rrs", [H, T]),
                          ("ath", [128, H * T]), ("sch", [128, H * T]),
                          ("or1", [65, T]), ("bc1", [64, T]),
                          ("z0", [128, T]), ("x2", [128, F6 * T])]:
            dbg_d[nm] = nc.dram_tensor("dbg_" + nm, shape, F32,
                                       kind="ExternalOutput")

    KELEM = 128 * F6 * T          # k shard elems per core (196608)
    VELEM = T * E                 # v shard elems per core (196608)
    AGE = KELEM + VELEM           # merged K+V AllGather payload (fp16)

    from contextlib import ExitStack
    with tile.TileContext(nc) as tc, ExitStack() as _es:
        constp = _es.enter_context(tc.tile_pool(name="const", bufs=1))
        residp = _es.enter_context(tc.tile_pool(name="resid", bufs=1))
        actp = _es.enter_context(tc.tile_pool(name="act", bufs=2))
        kvp = _es.enter_context(tc.tile_pool(name="kv", bufs=1))
        attnp = _es.enter_context(tc.tile_pool(name="attn", bufs=1))
        smallp = _es.enter_context(tc.tile_pool(name="small", bufs=2))
        wpool = _es.enter_context(tc.tile_pool(name="wpool", bufs=6))
        w2pool = _es.enter_context(tc.tile_pool(name="w2pool", bufs=2))
        mw2pool = _es.enter_context(tc.tile_pool(name="mw2pool", bufs=1))
        zpool = _es.enter_context(tc.tile_pool(name="zpool", bufs=16))
        psmm = _es.enter_context(tc.tile_pool(name="psmm", bufs=3, space="PSUM"))
        pssc = _es.enter_context(tc.tile_pool(name="pssc", bufs=2, space="PSUM"))
        pspv = _es.enter_context(tc.tile_pool(name="pspv", bufs=2, space="PSUM"))
        psh = _es.enter_context(tc.tile_pool(name="psh", bufs=1, space="PSUM"))
        dramp = _es.enter_context(tc.tile_pool(name="dram", bufs=2, space="DRAM"))

        # ---- constants ----
        ident = constp.tile([128, 128], F32)
        nc.sync.dma_start(out=ident, in_=id_d.ap())
        btab = constp.tile([128, KT * H], F32)
        nc.sync.dma_start(out=btab, in_=btab_d.ap())
        inv_e_bf = constp.tile([128, 1], BF16)
        nc.vector.memset(inv_e_bf, 1.0 / E)
        ones_f = constp.tile([128, 128], F32)
        nc.vector.memset(ones_f, 1.0)
        ones_row_bf = constp.tile([1, 128], BF16)
        nc.vector.memset(ones_row_bf, 1.0)
        eps_t = constp.tile([128, 1], F32)
        nc.vector.memset(eps_t, EPS)
        lng = constp.tile([128, n_ln * F6], F32)
        nc.sync.dma_start(out=lng, in_=lng_d.ap())
        lnb = constp.tile([128, n_ln * F6], F32)
        nc.sync.dma_start(out=lnb, in_=lnb_d.ap())
        qkb = constp.tile([128, 2 * F6], F32)
        nc.sync.dma_start(out=qkb, in_=qkb_d.ap())
        vb = constp.tile([1, E], F32)
        nc.sync.dma_start(out=vb, in_=vb_d.ap())
        ob = constp.tile([128, F6], F32)
        nc.sync.dma_start(out=ob, in_=ob_d.ap())
        f1b = constp.tile([128, 2 * FFN // 128], F32)
        nc.sync.dma_start(out=f1b, in_=f1b_d.ap())
        f2b = constp.tile([128, F6], F32)
        nc.sync.dma_start(out=f2b, in_=f2b_d.ap())
        m1b = constp.tile([128, F6], F32)
        nc.sync.dma_start(out=m1b, in_=m1b_d.ap())
        m2b = constp.tile([1, V], F32)
        nc.sync.dma_start(out=m2b, in_=m2b_d.ap())

        def tap(nm, ap):
            if DEBUG and nm in dbg_d:
                shp = list(ap.shape)
                mid = int(np.prod(shp[1:-1])) if len(shp) > 2 else 1
                pad = shp[:-1] + [max(shp[-1], -(-(F6 * T) // mid))]
                t = smallp.tile(shp, F32, tag="dbgscratch", bufs=1,
                                padded_shape=pad)
                nc.vector.tensor_copy(out=t, in_=ap)
                nc.sync.dma_start(out=dbg_d[nm].ap().rearrange(
                    "p (a b) -> p a b", a=ap.shape[1]) if len(ap.shape) == 3
                    else dbg_d[nm].ap(), in_=t)

        # ---- residual stream ----
        x = residp.tile([128, F6, T], F32)

        for _rep in range(reps):
            # ---- embedding gather + transpose to feature-major ----
            tokt = constp.tile([128, TT], I32)
            nc.sync.dma_start(out=tokt,
                              in_=tok_d.ap().rearrange("(a p) o -> p (a o)", p=128))
            for tt in range(TT):
                xtm = actp.tile([128, E], F32, tag="xtm", bufs=1)
                nc.gpsimd.indirect_dma_start(
                    out=xtm[:], out_offset=None, in_=emb_d.ap(),
                    in_offset=bass.IndirectOffsetOnAxis(ap=tokt[:, tt:tt + 1], axis=0))
                for f in range(F6):
                    tp = psmm.tile([128, 128], F32, tag="mm")
                    nc.tensor.transpose(out=tp[:], in_=xtm[:, f * 128:(f + 1) * 128],
                                        identity=ident[:])
                    nc.vector.tensor_copy(out=x[:, f, tt * 128:(tt + 1) * 128],
                                          in_=tp[:])

            # ---------------- helpers ----------------
            def layernorm(src, h_dst, ln_idx):
                """src fp32 [128,F6,T] -> h_dst bf16 [128,F6,T] (LN over features)."""
                g_ap = lng[:, ln_idx * F6:(ln_idx + 1) * F6]
                b_ap = lnb[:, ln_idx * F6:(ln_idx + 1) * F6]
                # cat holds [x | x^2] side by side so one matmul per ftile
                # produces both sums; inv_e lhsT folds the 1/E scaling.
                cat = kvp.tile([128, F6, 2, T], BF16, tag="ln_xb")
                nc.vector.tensor_copy(out=cat[:, :, 0, :], in_=src)
                nc.vector.tensor_tensor(out=cat[:, :, 1, :], in0=cat[:, :, 0, :],
                                        in1=cat[:, :, 0, :], op=ALU.mult)
                psAB = pssc.tile([1, 2 * T], F32, tag="sc")
                for k in range(F6):
                    nc.tensor.matmul(psAB[:], inv_e_bf[:, 0:1], cat[:, k, :, :],
                                     start=(k == 0), stop=(k == F6 - 1))
                st = smallp.tile([1, 4 * T], F32, tag="ln_st", bufs=1)
                # slots: 0 m, 1 ms/var/sqrt, 2 rstd, 3 mrstd  (2,3 adjacent for bcast)
                sl = lambda i: st[:, i * T:(i + 1) * T]
                nc.vector.tensor_copy(out=st[:, 0:2 * T], in_=psAB[:])
                nc.vector.tensor_tensor(out=sl(3), in0=sl(0), in1=sl(0), op=ALU.mult)
                nc.vector.tensor_tensor(out=sl(1), in0=sl(1), in1=sl(3),
                                        op=ALU.subtract)          # var
                nc.scalar.activation(out=sl(1), in_=sl(1), func=AF.Sqrt,
                                     bias=eps_t[0:1, 0:1])
                nc.vector.reciprocal_approx_fast(out=sl(2), in_=sl(1))   # rstd
                nc.vector.tensor_tensor(out=sl(3), in0=sl(0), in1=sl(2), op=ALU.mult)
                bc = pssc.tile([128, 2 * T], F32, tag="sc")
                nc.tensor.matmul(bc[:], ones_f[0:1, 0:128], st[0:1, 2 * T:4 * T],
                                 start=True, stop=True)
                for f in range(F6):
                    tmp = smallp.tile([128, T], F32, tag="ln_tmp")
                    nc.vector.tensor_tensor(out=tmp, in0=src[:, f, :],
                                            in1=bc[:, 0:T], op=ALU.mult)
                    nc.vector.tensor_tensor(out=h_dst[:, f, :], in0=tmp,
                                            in1=bc[:, T:2 * T], op=ALU.subtract)
                    if ln_affine:
                        nc.vector.tensor_scalar(
                            out=h_dst[:, f, :], in0=h_dst[:, f, :],
                            scalar1=g_ap[:, f:f + 1], scalar2=b_ap[:, f:f + 1],
                            op0=ALU.mult, op1=ALU.add)

            def linear_fm(h_bf, w_dram, mtiles, out_cb, bias_tile=None,
                          w_tag="w"):
                """out[m] = W.T-tile-m @ h (feature-major out), psum -> out_cb."""
                for m in range(mtiles):
                    wt = wpool.tile([128, F6, 128], BF16, tag=w_tag)
                    nc.sync.dma_start(
                        out=wt,
                        in_=w_dram.ap().rearrange("(a p) n -> p a n", p=128)[
                            :, :, m * 128:(m + 1) * 128])
                    ps = psmm.tile([128, T], F32, tag="mm")
                    for k in range(F6):
                        nc.tensor.matmul(ps[:], wt[:, k, :], h_bf[:, k, :],
                                         start=(k == 0), stop=(k == F6 - 1))
                    out_cb(m, ps, bias_tile)

            tap("x0", x[:])
            # ---------------- layers ----------------
            for i in range(NL):
                h_bf = actp.tile([128, F6, T], BF16, tag="h")
                layernorm(x, h_bf, 2 * i)
                if i == 0:
                    tap("h0", h_bf[:])

                # --- qkv: K first so its AllGather launches ASAP ---
                q_bf = actp.tile([128, F6, T], BF16, tag="q", bufs=1)
                kshard = actp.tile([128, F6, T], BF16, tag="kshard", bufs=1)
                wqkr = wqk_d[i].ap().rearrange("(a p) n -> p a n", p=128)

                def qk_mm(m, dst):
                    wt = wpool.tile([128, F6, 128], BF16, tag="w")
                    nc.sync.dma_start(out=wt,
                                      in_=wqkr[:, :, m * 128:(m + 1) * 128])
                    ps = psmm.tile([128, T], F32, tag="mm")
                    for k in range(F6):
                        nc.tensor.matmul(ps[:], wt[:, k, :], h_bf[:, k, :],
                                         start=(k == 0), stop=(k == F6 - 1))
                    if has_qkb:
                        nc.vector.tensor_scalar_add(out=dst, in0=ps[:],
                                                    scalar1=qkb[:, m:m + 1])
                    else:
                        nc.vector.tensor_copy(out=dst, in_=ps[:])

                for m in range(F6, 2 * F6):
                    qk_mm(m, kshard[:, m - F6, :])

                ag_in = dramp.tile([1, AGE], BF16, tag="ag_in")
                ag_out = dramp.tile([NC, AGE], BF16, tag="ag_out",
                                    addr_space="Shared")
                nc.sync.dma_start(
                    out=ag_in[0:1, 0:KELEM].rearrange("o (p f t) -> (o p) f t",
                                                      p=128, f=F6),
                    in_=kshard)

                # --- V projection (packed into the same AllGather as K) ---
                vshard = actp.tile([128, TT, E], BF16, tag="vshard", bufs=1)
                wvt = kvp.tile([128, F6, E], BF16, tag="wv")
                nc.sync.dma_start(out=wvt,
                                  in_=wv_d[i].ap().rearrange("(a p) n -> p a n", p=128))
                for tt in range(TT):
                    for n0 in (0, 384):
                        ps = psmm.tile([128, 384], F32, tag="mm")
                        for k in range(F6):
                            nc.tensor.matmul(ps[:], h_bf[:, k, tt * 128:(tt + 1) * 128],
                                             wvt[:, k, n0:n0 + 384],
                                             start=(k == 0), stop=(k == F6 - 1))
                        if has_vb:
                            bcv = pssc.tile([128, 384], F32, tag="sc")
                            nc.tensor.matmul(bcv[:], ones_f[0:1, :],
                                             vb[0:1, n0:n0 + 384], start=True, stop=True)
                            nc.vector.tensor_tensor(out=vshard[:, tt, n0:n0 + 384],
                                                    in0=ps[:], in1=bcv[:], op=ALU.add)
                        else:
                            nc.vector.tensor_copy(out=vshard[:, tt, n0:n0 + 384],
                                                  in_=ps[:])
                if i == 0:
                    tap("v0", vshard[:])
                nc.sync.dma_start(
                    out=ag_in[0:1, KELEM:AGE].rearrange(
                        "o (tt p e) -> (o p) tt e", tt=TT, p=128),
                    in_=vshard)
                if NO_AG:
                    nc.sync.dma_start(out=ag_out[0:1, :], in_=ag_in[:])
                else:
                    nc.gpsimd.collective_compute(
                        "AllGather", ALU.bypass,
                        replica_groups=[list(range(NC))],
                        ins=[ag_in[:]], outs=[ag_out[:]])
                k_sb = kvp.tile([128, F6, L], BF16, tag="k_sb")
                # load only the key-tile tail ranges any head of this
                # feature-pair keeps (ALiBi tile skipping)
                ksrc = ag_out[:, 0:KELEM].rearrange("r (p f t) -> p f r t",
                                                    p=128, f=F6)
                kdst = k_sb.rearrange("p f (r t) -> p f r t", r=NC)
                for f0 in range(F6):
                    kmin = min(kept_ktiles(2 * f0)[0], kept_ktiles(2 * f0 + 1)[0])
                    r0 = kmin // TT
                    nc.sync.dma_start(out=kdst[:, f0:f0 + 1, r0:NC, :],
                                      in_=ksrc[:, f0:f0 + 1, r0:NC, :])
                vhat = kvp.tile([128, KT, H * 65], BF16, tag="vhat")
                VROW = H * 65  # 780 elems per (partition, ktile)
                for kt in range(KT):
                    r, tt = kt // TT, kt % TT
                    off = tt * (128 * E)
                    # heads that keep this key tile, as contiguous runs
                    hs = [h for h in range(H) if kept_ktiles(h)[0] <= kt]
                    runs = []
                    for h in hs:
                        if runs and runs[-1][1] == h - 1:
                            runs[-1][1] = h
                        else:
                            runs.append([h, h])
                    vsrc = ag_out[r:r + 1, KELEM + off:KELEM + off + 128 * E
                                  ].rearrange("r (p h d) -> (r p) h d", p=128, h=H)
                    for h0, h1 in runs:
                        nh = h1 - h0 + 1
                        nc.sync.dma_start(
                            out=bass.AP(tensor=vhat.tensor,
                                        offset=vhat.offset + kt * VROW + h0 * 65,
                                        ap=[vhat.ap[0],
                                            [65, nh],           # h
                                            [1, 64]]),          # d
                            in_=vsrc[:, h0:h1 + 1, :])
                # ones columns (col 64 of each head block)
                nc.vector.memset(
                    bass.AP(tensor=vhat.tensor, offset=vhat.offset + 64,
                            ap=[vhat.ap[0], [VROW, KT], [65, H]]),
                    1.0)

                # --- Q projection (overlaps the AllGathers) ---
                for m in range(F6):
                    qk_mm(m, q_bf[:, m, :])
                if i == 0:
                    tap("q0", q_bf[:])
                    tap("k0", kshard[:])
                if i == 0:
                    tap("ksb", k_sb[:, :, 0:128])
                    tap("vhb", vhat[:, 0:2, :])
                # --- attention: QK+exp for ALL heads first (overlaps the V
                # AllGather), then all PV+normalize ---
                o_bf = actp.tile([128, F6, T], BF16, tag="o", bufs=1)
                attnTs = []
                for h in range(H):
                    f0, r0 = h // 2, (h % 2) * 64
                    kts = kept_ktiles(h)
                    attnT = attnp.tile([128, len(kts), T], BF16,
                                       tag=f"attnT{h}", name=f"attnT{h}")
                    for g0 in range(0, len(kts), 2):
                        grp = kts[g0:g0 + 2]
                        sc = pssc.tile([128, len(grp) * T], F32, tag="sc")
                        for j, kt in enumerate(grp):
                            nc.tensor.matmul(
                                sc[:, j * T:(j + 1) * T],
                                k_sb[r0:r0 + 64, f0, kt * 128:(kt + 1) * 128],
                                q_bf[r0:r0 + 64, f0, :],
                                start=True, stop=True)
                            if DEBUG and i == 0 and kt == KT - 1:
                                scht = smallp.tile([128, T], F32, tag="scht")
                                nc.vector.tensor_copy(out=scht,
                                                      in_=sc[:, j * T:(j + 1) * T])
                                nc.sync.dma_start(
                                    out=dbg_d["sch"].ap()[:, h * T:(h + 1) * T],
                                    in_=scht)
                            nc.scalar.activation(
                                out=attnT[:, g0 + j, :], in_=sc[:, j * T:(j + 1) * T],
                                func=AF.Exp, bias=btab[:, kt * H + h:kt * H + h + 1],
                                scale=SCALE)
                    if DEBUG and i == 0:
                        atht = smallp.tile([128, T], F32, tag="atht")
                        nc.vector.tensor_copy(out=atht, in_=attnT[:, len(kts) - 1, :])
                        nc.sync.dma_start(out=dbg_d["ath"].ap()[:, h * T:(h + 1) * T],
                                          in_=atht)
                    attnTs.append((attnT, kts))
                for h in range(H):
                    f0, r0 = h // 2, (h % 2) * 64
                    attnT, kts = attnTs[h]
                    if i == 0 and h == 0 and len(kts) >= 2 and kts[0] == 0:
                        tap("at0", attnT[:, 0:2, :])
                    pv = pspv.tile([65, T], F32, tag="pv")
                    for n, kt in enumerate(kts):
                        nc.tensor.matmul(pv[:], vhat[:, kt, h * 65:h * 65 + 65],
                                         attnT[:, n, :],
                                         start=(n == 0), stop=(n == len(kts) - 1))
                    o_raw = smallp.tile([65, T], F32, tag="o_raw")
                    nc.scalar.activation(out=o_raw, in_=pv[:], func=AF.Copy)
                    srow = smallp.tile([1, T], F32, tag="srow")
                    nc.vector.tensor_copy(out=srow[0:1, :], in_=o_raw[64:65, :])
                    rr = smallp.tile([1, T], F32, tag="rr")
                    nc.vector.reciprocal_approx_fast(out=rr[0:1, :],
                                                     in_=srow[0:1, :])
                    bc = psmm.tile([64, T], F32, tag="mm")
                    nc.tensor.matmul(bc[:], ones_f[0:1, 0:64], rr[0:1, :],
                                     start=True, stop=True)
                    if DEBUG and i == 0:
                        nc.sync.dma_start(out=dbg_d["sums"].ap()[h:h + 1, :],
                                          in_=o_raw[64:65, :])
                        nc.sync.dma_start(out=dbg_d["rrs"].ap()[h:h + 1, :],
                                          in_=rr[0:1, :])
                        if h == 1:
                            nc.sync.dma_start(out=dbg_d["or1"].ap(), in_=o_raw[:])
                            bc1t = smallp.tile([64, T], F32, tag="bc1t", bufs=1)
                            nc.vector.tensor_copy(out=bc1t, in_=bc[:])
                            nc.sync.dma_start(out=dbg_d["bc1"].ap(), in_=bc1t)
                    nc.vector.tensor_tensor(out=o_bf[r0:r0 + 64, f0, :],
                                            in0=o_raw[0:64, :], in1=bc[:],
                                            op=ALU.mult)
                # --- out_proj + residual ---
                def oproj_cb(m, ps, _):
                    if has_ob:
                        tmp2 = smallp.tile([128, T], F32, tag="ob_tmp")
                        nc.vector.tensor_scalar_add(out=tmp2, in0=ps[:],
                                                    scalar1=ob[:, m:m + 1])
                        nc.vector.tensor_tensor(out=x[:, m, :], in0=x[:, m, :],
                                                in1=tmp2, op=ALU.add)
                    else:
                        nc.vector.tensor_tensor(out=x[:, m, :], in0=x[:, m, :],
                                                in1=ps[:], op=ALU.add)

                linear_fm(o_bf, wo_d[i], F6, oproj_cb, w_tag="w")

                if i == 0:
                    tap("x1", x[:])
                # --- ffn ---
                h2 = actp.tile([128, F6, T], BF16, tag="h")
                layernorm(x, h2, 2 * i + 1)
                w1r = w1_d[i].ap().rearrange("(a p) n -> p a n", p=128)
                w2r = w2_d[i].ap().rearrange("(a p) n -> p a n", p=128)
                zs = []
                for j in range(FFN // 128):
                    wa = wpool.tile([128, F6, 128], BF16, tag="w")
                    nc.sync.dma_start(out=wa, in_=w1r[:, :, (2 * j) * 128:(2 * j + 1) * 128])
                    wg = wpool.tile([128, F6, 128], BF16, tag="w")
                    nc.sync.dma_start(out=wg, in_=w1r[:, :, (2 * j + 1) * 128:(2 * j + 2) * 128])
                    pa = psmm.tile([128, T], F32, tag="mm")
                    pg = psmm.tile([128, T], F32, tag="mm")
                    for k in range(F6):
                        nc.tensor.matmul(pa[:], wa[:, k, :], h2[:, k, :],
                                         start=(k == 0), stop=(k == F6 - 1))
                    for k in range(F6):
                        nc.tensor.matmul(pg[:], wg[:, k, :], h2[:, k, :],
                                         start=(k == 0), stop=(k == F6 - 1))
                    gg = smallp.tile([128, T], F32, tag="gg")
                    if has_f1b:
                        nc.vector.tensor_scalar_add(out=pa[:], in0=pa[:],
                                                    scalar1=f1b[:, 2 * j:2 * j + 1])
                        nc.scalar.activation(out=gg, in_=pg[:], func=AF.Gelu,
                                             bias=f1b[:, 2 * j + 1:2 * j + 2])
                    else:
                        nc.scalar.activation(out=gg, in_=pg[:], func=AF.Gelu)
                    z = zpool.tile([128, T], BF16, tag="z")
                    nc.vector.tensor_tensor(out=z, in0=pa[:], in1=gg, op=ALU.mult)
                    zs.append(z)
                if i == 0:
                    tap("z0", zs[0][:])
                for f in range(F6):
                    w2f = w2pool.tile([128, FFN // 128, 128], BF16, tag="w2")
                    nc.sync.dma_start(out=w2f, in_=w2r[:, :, f * 128:(f + 1) * 128])
                    ps = psmm.tile([128, T], F32, tag="mm")
                    for j in range(FFN // 128):
                        nc.tensor.matmul(ps[:], w2f[:, j, :], zs[j][:],
                                         start=(j == 0), stop=(j == FFN // 128 - 1))
                    if has_f2b:
                        tmp3 = smallp.tile([128, T], F32, tag="f2b_tmp")
                        nc.vector.tensor_scalar_add(out=tmp3, in0=ps[:],
                                                    scalar1=f2b[:, f:f + 1])
                        nc.vector.tensor_tensor(out=x[:, f, :], in0=x[:, f, :],
                                                in1=tmp3, op=ALU.add)
                    else:
                        nc.vector.tensor_tensor(out=x[:, f, :], in0=x[:, f, :],
                                                in1=ps[:], op=ALU.add)

            tap("x2", x[:])
            # ---------------- mlm head ----------------
            hf = actp.tile([128, F6, T], BF16, tag="h")
            layernorm(x, hf, 2 * NL)
            g1 = residp.tile([128, F6, T], F32)

            def mlm1_cb(m, ps, _):
                if has_m1b:
                    nc.scalar.activation(out=g1[:, m, :], in_=ps[:], func=AF.Gelu,
                                         bias=m1b[:, m:m + 1])
                else:
                    nc.scalar.activation(out=g1[:, m, :], in_=ps[:], func=AF.Gelu)

            linear_fm(hf, mw1_d, F6, mlm1_cb, w_tag="w")

            h2f = actp.tile([128, F6, T], BF16, tag="h")
            layernorm(g1, h2f, 2 * NL + 1)

            mw2r = mw2_d.ap().rearrange("(a p) n -> p a n", p=128)
            for vc in range(V // 512):
                wt = mw2pool.tile([128, F6, 512], BF16, tag="mw2")
                nc.sync.dma_start(out=wt, in_=mw2r[:, :, vc * 512:(vc + 1) * 512])
                for tt in range(TT):
                    ps = psh.tile([128, 512], F32, tag="mmh")
                    for k in range(F6):
                        nc.tensor.matmul(ps[:], h2f[:, k, tt * 128:(tt + 1) * 128],
                                         wt[:, k, :],
                                         start=(k == 0),
                                         stop=(k == F6 - 1 and not has_m2b))
                    if has_m2b:
                        m2bb = smallp.tile([1, 512], BF16, tag="m2bb")
                        nc.vector.tensor_copy(out=m2bb,
                                              in_=m2b[0:1, vc * 512:(vc + 1) * 512])
                        nc.tensor.matmul(ps[:], ones_row_bf[0:1, :],
                                         m2bb[0:1, :], start=False, stop=True)
                    osb = smallp.tile([128, 512], BF16, tag="osb")
                    nc.vector.tensor_copy(out=osb, in_=ps[:])
                    nc.sync.dma_start(
                        out=out_d.ap()[tt * 128:(tt + 1) * 128, vc * 512:(vc + 1) * 512],
                        in_=osb)

    nc.compile()
    _CACHE[key] = nc
    return nc


# ---------------------------------------------------------------------------
# host wrapper
# ---------------------------------------------------------------------------

def _host_pack(inputs):
    """Build the shared (core-independent) input arrays."""
    d = {}
    d["emb"] = np.ascontiguousarray(inputs["embed"], dtype=np.float32)
    d["ident"] = np.eye(128, dtype=np.float32)

    # alibi column-bias table [128, KT*H]: b[p, kt*H+h] = s_h*((kt*128+p)-(L-1)) + pad
    mask = np.asarray(inputs["attention_mask"]).reshape(L)
    pad = (mask == 0).astype(np.float32)           # reference adds +1.0 float mask
    j = np.arange(L, dtype=np.float32)
    # +ln(32) rescales the exp outputs into fp8e4m3's normal range; the
    # denominator row of v-hat scales identically so softmax cancels it.
    colb = (SLOPES[None, :] * (j[:, None] - (L - 1)) + pad[:, None]
            + math.log(32.0))                                        # [L, H]
    d["btab"] = np.ascontiguousarray(
        colb.reshape(KT, 128, H).transpose(1, 0, 2).reshape(128, KT * H)
    ).astype(np.float32)

    in_w = np.asarray(inputs["in_w"], dtype=np.float32)    # [NL, 3E, E]
    in_b = np.asarray(inputs["in_b"], dtype=np.float32)
    out_w = np.asarray(inputs["out_w"], dtype=np.float32)
    ffn_w1 = np.asarray(inputs["ffn_w1"], dtype=np.float32)
    ffn_w2 = np.asarray(inputs["ffn_w2"], dtype=np.float32)
    for i in range(NL):
        wqk = in_w[i, :2 * E].T.copy()                     # [E, 2E]
        d[f"wqk{i}"] = bf(wqk)
        d[f"wv{i}"] = bf(in_w[i, 2 * E:].T)                # [E, E] rhs layout
        d[f"wo{i}"] = bf(out_w[i].T)
        w1t = ffn_w1[i].T.reshape(E, 2, FFN // 128, 128)   # [E][a/g][j][128]
        w1t = w1t.transpose(0, 2, 1, 3).reshape(E, 2 * FFN)  # interleave a0 g0 a1 g1
        d[f"w1{i}"] = bf(w1t)
        d[f"w2{i}"] = bf(ffn_w2[i].T)                      # [FFN, E]
    d["mw1"] = bf(np.asarray(inputs["mlm_w1"], dtype=np.float32).T)
    d["mw2"] = bf(np.asarray(inputs["mlm_w2"], dtype=np.float32).T)   # [E, V]

    def pack_pf(vec):   # [E] -> [128, F6] feature-major per-partition
        return np.ascontiguousarray(
            np.asarray(vec, dtype=np.float32).reshape(F6, 128).T)

    lng, lnb = [], []
    for i in range(NL):
        lng.append(pack_pf(inputs["norm1_g"][i])); lnb.append(pack_pf(inputs["norm1_b"][i]))
        lng.append(pack_pf(inputs["ffn_g"][i])); lnb.append(pack_pf(inputs["ffn_bt"][i]))
    lng.append(pack_pf(inputs["fin_g"])); lnb.append(pack_pf(inputs["fin_b"]))
    lng.append(pack_pf(inputs["mlm_g"])); lnb.append(pack_pf(inputs["mlm_bt"]))
    d["lng"] = np.concatenate(lng, axis=1)
    d["lnb"] = np.concatenate(lnb, axis=1)

    qb = np.asarray(inputs["in_b"], dtype=np.float32)
    qkb = np.zeros((128, 2 * F6), np.float32)
    # note: per-layer biases differ; only support layer-invariant zero biases
    # in the fused path. If any nonzero, fall back handled via flags (we pack
    # layer 0's; correctness enforced by flag check in kernel()).
    qkvec = qb[0, :2 * E].copy()
    qkb[:, :] = qkvec.reshape(2 * F6, 128).T
    d["qkb"] = qkb
    d["vb"] = qb[0, 2 * E:].reshape(1, E).copy()
    d["ob"] = pack_pf(np.asarray(inputs["out_b"], dtype=np.float32)[0])
    f1 = np.asarray(inputs["ffn_b1"], dtype=np.float32)[0]
    f1r = f1.reshape(2, FFN // 128, 128).transpose(1, 0, 2).reshape(2 * FFN)
    d["f1b"] = np.ascontiguousarray(f1r.reshape(2 * FFN // 128, 128).T)
    d["f2b"] = pack_pf(np.asarray(inputs["ffn_b2"], dtype=np.float32)[0])
    d["m1b"] = pack_pf(np.asarray(inputs["mlm_b1"], dtype=np.float32))
    d["m2b"] = np.asarray(inputs["mlm_b2"], dtype=np.float32).reshape(1, V).copy()
    return d


def kernel(**inputs):
    shared = _host_pack(inputs)
    tokens = np.asarray(inputs["tokens"]).reshape(L)

    def nz(a):
        return bool(np.any(np.asarray(a) != 0))

    ln_affine = (nz(np.asarray(inputs["norm1_g"]) - 1) or nz(inputs["norm1_b"])
                 or nz(np.asarray(inputs["ffn_g"]) - 1) or nz(inputs["ffn_bt"])
                 or nz(np.asarray(inputs["fin_g"]) - 1) or nz(inputs["fin_b"])
                 or nz(np.asarray(inputs["mlm_g"]) - 1) or nz(inputs["mlm_bt"]))
    flags = (ln_affine,
             nz(inputs["in_b"][:, :2 * E]), nz(inputs["in_b"][:, 2 * E:]),
             nz(inputs["out_b"]), nz(inputs["ffn_b1"]), nz(inputs["ffn_b2"]),
             nz(inputs["mlm_b1"]), nz(inputs["mlm_b2"]))
    if any(flags[1:]) :
        # per-layer bias tensors packed only for layer 0; replicate properly
        # (all-zero in the reference problem, so this path is never hot)
        assert all(
            np.array_equal(np.asarray(inputs[k])[0], np.asarray(inputs[k])[j])
            for k in ("in_b", "out_b", "ffn_b1", "ffn_b2") for j in range(NL)
        ), "per-layer biases differing across layers not supported"

    nc = build(flags)

    in_maps = []
    for c in range(NC):
        m = dict(shared)
        m["tok"] = np.ascontiguousarray(
            tokens[c * T:(c + 1) * T].reshape(T, 1).astype(np.int32))
        in_maps.append(m)

    res = bass_utils.run_bass_kernel_spmd(
        nc, in_maps, core_ids=list(range(NC)), trace=TRACE)
    out = np.concatenate([res.results[c]["out"] for c in range(NC)], axis=0)
    kernel.last_result = res
    return out.astype(np.float32).reshape(B, L, V)

